# revision 48
# baseline (speedup 1.0000x reference)
"""Bass/Tile TRN2 kernel for nn_DecoderGroupedQueryHeadAttentionAlibi.

Sharding (8 cores): core = (b, g) with b = core//2 in [0,4) (batch),
g = core%2 (head-half). Each core computes 8 of 16 query heads (those with
h%4 in {2g, 2g+1}) for its batch, plus the corresponding row-slice of the
output projection; the host sums the two half partials and adds bproj.

v3 changes over the baseline:
  - the j-loop is software-pipelined: attn@v for s-tile j is emitted after
    scores/exp/multipliers of s-tile j+1, so the PE streams the next tile's
    scores while the DVE finishes the current tile's row multipliers
    (removes a per-tile PE stall on the DVE).
  - x input DMA is split into t-halves so the first projections start
    after half the input has landed.
  - psum attn accumulator is split into two [65,1024] tiles so the next
    head's attn@v can start after half the epilogue copy.
  - per-head epilogue: direct [1,T] reciprocal + one DRAM-broadcast hop
    (replaces a 4-hop DMA round-trip chain).
  - output projection reuses the score PSUM pool (no pool-transition
    barrier), writes bf16 (summed on host in fp32), pipelines per t-tile,
    and spreads the output DMA over 3 queues.

Per-core device program (layout A, scoresT = [s_partitions, t_free]):
  per (head, s-tile): scoresT psum [128,1024] tiles -> ACT exp (alibi bias
  folded into the per-partition activation bias) -> DVE multiplier applied
  per region (past/diag/future) -> attn@v psum accumulation where row 64
  (a ones column in v) is the softmax denominator.

The alibi bias of this module is min(a_h*(s-t), 0) (tril overwrites the
causal mask in the torch reference, so future tokens are attended with bias
0), hence P = exp(score/8) * min(exp(a*(s-t)), 1), which factors into a
per-partition ACT bias exp(a*(s_in-127)) and a distance-only (Toeplitz) row
multiplier exp(-a*delta). Score columns with t - s > margin/a are dropped
(banded): the dropped softmax mass is < e^-margin of the kept band mass.
"""

import math
import numpy as np

# ---- problem constants (hardcoded; kernel.py must be self-contained) ----
B, T, C = 4, 2048, 1024
N_HEAD, N_KV_HEAD, HEAD_DIM = 16, 4, 64
NH = 8            # heads per core
ST = T // 128     # 16 s-tiles
NCH = T // 512    # 4 t-chunks
KCT = C // 128    # 8 contraction tiles of 128
WREP_W = 2048     # Toeplitz table width: index = t - 128*j is always < 2048
MARGIN = 4.5      # exp(-4.5): adds ~3e-3 relmax (validated), total ~9e-3 < 2e-2

_START = 2.0 ** (-2.0 ** (-(math.log2(N_HEAD) - 3.0)))  # 0.7071...


def _head_of_slot(i: int, g: int) -> int:
    return 4 * (i // 2) + 2 * g + (i % 2)


def _a_of_head(h: int) -> float:
    return (_START ** (h + 1)) / math.sqrt(HEAD_DIM)


# Loop bounds must be identical on every core (SPMD): use the widest cutoff
# over g for each head slot (g=1 heads have smaller slopes -> wider bands).
_CUTOFF = [MARGIN / min(_a_of_head(_head_of_slot(i, 0)),
                        _a_of_head(_head_of_slot(i, 1)))
           for i in range(NH)]
_R128 = [min(T, int(math.ceil(c / 128.0)) * 128) for c in _CUTOFF]
# 128-granular computed width per (head slot, s-tile): scores/exp/multiplies
_W128 = [[min(T, 128 * (j + 1) + _R128[i]) for j in range(ST)]
         for i in range(NH)]
# 512-granular width for the attn@v accumulation (E tail is zeroed)
_W512 = [[((w + 511) // 512) * 512 for w in row] for row in _W128]
_NEFF = [[w // 512 for w in row] for row in _W512]
_J_FIRST = [[min(j for j in range(ST) if _NEFF[i][j] > tcn)
             for tcn in range(NCH)] for i in range(NH)]

_NC_CACHE = {}


def _split_multiwait(nc, mybir, max_waits=1):
    """walrus in this env encodes at most one sync-wait per instruction;
    split extras onto same-engine NoOps emitted just before."""
    for f in nc.m.functions:
        for bb in f.blocks:
            new = []
            for ins in bb.instructions:
                si = ins.sync_info
                conds = list(si.on_wait) if si is not None else []
                if len(conds) > max_waits:
                    for cond in conds[:-max_waits]:
                        n = mybir.InstNoOp(
                            name=nc.get_next_instruction_name(), ins=[], outs=[])
                        n.engine = ins.engine
                        n.sync_info = mybir.SyncInfo(on_wait=[cond], on_update=[])
                        new.append(n)
                    si.on_wait = conds[-max_waits:]
                new.append(ins)
            bb.instructions = new


def _build_nc():
    if "nc" in _NC_CACHE:
        return _NC_CACHE["nc"]
    import concourse.bass as bass
    import concourse.tile as tile
    from concourse import mybir

    f32 = mybir.dt.float32
    bf16 = mybir.dt.bfloat16
    AF = mybir.ActivationFunctionType
    MUL = mybir.AluOpType.mult
    MIN = mybir.AluOpType.min

    nc = bass.Bass()

    xT_d = nc.dram_tensor("xT", [C, T], bf16, kind="ExternalInput")
    wq_d = nc.dram_tensor("wqT", [C, NH * 64], bf16, kind="ExternalInput")
    wk_d = nc.dram_tensor("wkT", [C, 128], bf16, kind="ExternalInput")
    wv_d = nc.dram_tensor("wvT", [C, 128], bf16, kind="ExternalInput")
    wp_d = nc.dram_tensor("wpT", [NH * 64, C], bf16, kind="ExternalInput")
    wrow_d = nc.dram_tensor("wrow", [NH, WREP_W], bf16, kind="ExternalInput")
    u_d = nc.dram_tensor("usb", [128, NH], f32, kind="ExternalInput")
    bias_d = nc.dram_tensor("biassb", [128, NH], f32, kind="ExternalInput")
    out_d = nc.dram_tensor("out", [T, C], bf16, kind="ExternalOutput")

    xT_r = xT_d.rearrange("(k p) t -> p k t", p=128)
    wq_r = wq_d.rearrange("(k p) e -> p k e", p=128)

    def bcast(src_row, parts):
        # [1, W] DRAM row -> [parts, W] stride-0 partition broadcast source
        return bass.AP(tensor=src_row.tensor, offset=src_row.offset,
                       ap=[[0, parts]] + list(src_row.ap)[1:])

    with tile.TileContext(nc) as tc:
        with (
            tc.tile_pool(name="const", bufs=1) as const,
            tc.tile_pool(name="work", bufs=3) as work,
            tc.tile_pool(name="ebuf", bufs=4) as ebufp,
            tc.tile_pool(name="stp", bufs=2) as stp,
            tc.tile_pool(name="rrp", bufs=2) as rrp,
            tc.tile_pool(name="dpk", bufs=4) as dpk,
            tc.tile_pool(name="outp", bufs=4) as outp,
            tc.tile_pool(name="dramd", bufs=1, space="DRAM") as dramd,
        ):
            # ---- persistent tiles ----
            kRep = const.tile([128, 2, T], bf16)     # kv on both halves
            v_sb = const.tile([128, ST, 130], bf16)  # [s, j, (v_kv0|1|v_kv1|1)]
            qRep = const.tile([128, NH, T], bf16)    # head i on both halves
            outT = const.tile([128, 4, T], bf16)     # [(2 heads d), pair, t]
            wrep = const.tile([128, NH, WREP_W], bf16)
            wp = const.tile([128, 4, C], bf16)
            usb = const.tile([128, NH], f32)
            biassb = const.tile([128, NH], f32)
            xT = const.tile([128, KCT, T], bf16)
            wq = const.tile([128, KCT, NH * 64], bf16)
            wk = const.tile([128, KCT, 128], bf16)
            wv = const.tile([128, KCT, 128], bf16)
            warm = const.tile([128, 1], f32)
            wsink = const.tile([128, 1], f32)
            drow_d = dramd.tile([NH, T], bf16)
            rrow_d = dramd.tile([NH, T], bf16)

            # ---- ACT exp-table preload (runs during the DMA ramp) ----
            nc.vector.memset(warm, 0.0)
            nc.scalar.activation(wsink, warm, AF.Exp, scale=1.0)

            # ---- input DMAs; x lands in t-order (first 512 cols of every
            # ---- contraction chunk first) so the projections start early
            for sc in range(2):
                for kc in range(KCT):
                    eng = nc.sync if kc % 2 == 0 else nc.scalar
                    eng.dma_start(out=xT[:, kc, 512 * sc:512 * (sc + 1)],
                                  in_=xT_r[:, kc, 512 * sc:512 * (sc + 1)])
            for kc in range(KCT):
                eng = nc.sync if kc % 2 == 0 else nc.scalar
                eng.dma_start(out=xT[:, kc, 1024:2048],
                              in_=xT_r[:, kc, 1024:2048])
            nc.gpsimd.dma_start(out=wk, in_=wk_d.rearrange("(k p) e -> p k e", p=128))
            nc.gpsimd.dma_start(out=usb, in_=u_d[:])
            nc.gpsimd.dma_start(out=biassb, in_=bias_d[:])
            wrow_r = [wrow_d[i:i + 1, :] for i in range(NH)]

            def wrep_bc(i):
                nc.gpsimd.dma_start(out=wrep[:, i, :], in_=bcast(wrow_r[i], 128))

            wrep_bc(0)
            wrep_bc(1)
            for kc in range(KCT):
                nc.gpsimd.dma_start(out=wq[:, kc, :], in_=wq_r[:, kc, :])
            nc.gpsimd.dma_start(out=wv, in_=wv_d.rearrange("(k p) e -> p k e", p=128))
            for i in range(2, NH):
                wrep_bc(i)
            nc.gpsimd.dma_start(out=wp, in_=wp_d.rearrange("(k p) e -> p k e", p=128))

            with (
                tc.tile_pool(name="psS", bufs=2, space="PSUM") as psS,
                tc.tile_pool(name="psA", bufs=1, space="PSUM") as psAp,
            ):
                def _copy(eng, out, in_):
                    if eng is nc.scalar:
                        eng.copy(out, in_)
                    else:
                        eng.tensor_copy(out, in_)

                # ---- projection emitters (share the psS psum pool) ----
                def k_proj_sh(sh):
                    ceng = nc.vector
                    ps = psS.tile([128, 1024], f32, tag="S", name=f"kp{sh}")
                    for sub in range(2):
                        sc = 2 * sh + sub
                        for kc in range(KCT):
                            nc.tensor.matmul(
                                ps[:, 512 * sub:512 * (sub + 1)],
                                lhsT=wk[:, kc, :],
                                rhs=xT[:, kc, 512 * sc:512 * (sc + 1)],
                                start=(kc == 0), stop=(kc == KCT - 1))
                    sl = slice(1024 * sh, 1024 * (sh + 1))
                    _copy(ceng, kRep[0:64, 0, sl], ps[0:64, :])
                    _copy(ceng, kRep[64:128, 1, sl], ps[64:128, :])
                    nc.sync.dma_start(out=kRep[64:128, 0, sl],
                                      in_=kRep[0:64, 0, sl])
                    nc.sync.dma_start(out=kRep[0:64, 1, sl],
                                      in_=kRep[64:128, 1, sl])

                def q_half(p, h):
                    ceng = nc.vector
                    ps = psS.tile([128, 1024], f32, tag="S", name=f"qp{p}{h}")
                    for sub in range(2):
                        tcn = 2 * h + sub
                        for kc in range(KCT):
                            nc.tensor.matmul(
                                ps[:, 512 * sub:512 * (sub + 1)],
                                lhsT=wq[:, kc, 128 * p:128 * (p + 1)],
                                rhs=xT[:, kc, 512 * tcn:512 * (tcn + 1)],
                                start=(kc == 0), stop=(kc == KCT - 1))
                    sl = slice(1024 * h, 1024 * (h + 1))
                    _copy(ceng, qRep[0:64, 2 * p, sl], ps[0:64, :])
                    _copy(ceng, qRep[64:128, 2 * p + 1, sl], ps[64:128, :])
                    nc.sync.dma_start(out=qRep[64:128, 2 * p, sl],
                                      in_=qRep[0:64, 2 * p, sl])
                    nc.sync.dma_start(out=qRep[0:64, 2 * p + 1, sl],
                                      in_=qRep[64:128, 2 * p + 1, sl])

                def v_half(h):
                    ceng = nc.vector
                    ps = psS.tile([128, 1024], f32, tag="S", name=f"vh{h}")
                    for b in range(8):
                        st = 8 * h + b
                        for kc in range(KCT):
                            nc.tensor.matmul(
                                ps[:, 128 * b:128 * (b + 1)],
                                lhsT=xT[:, kc, 128 * st:128 * (st + 1)],
                                rhs=wv[:, kc, :],
                                start=(kc == 0), stop=(kc == KCT - 1))
                    ps3 = ps.rearrange("p (s d) -> p s d", d=128)
                    sl = slice(8 * h, 8 * (h + 1))
                    _copy(ceng, v_sb[:, sl, 0:64], ps3[:, :, 0:64])
                    _copy(ceng, v_sb[:, sl, 65:129], ps3[:, :, 64:128])
                    nc.vector.memset(v_sb[:, sl, 64], 1.0)
                    nc.vector.memset(v_sb[:, sl, 129], 1.0)

                RR = {}
                DMIN = {}

                def emit_scores(i, j, mid=None):
                    p, half = i // 2, i % 2
                    W, W5 = _W128[i][j], _W512[i][j]
                    lo = 128 * j         # t < lo : future region
                    hi = 128 * (j + 1)   # t >= hi: past region (Toeplitz)
                    nchunks = (W + 511) // 512
                    E = ebufp.tile([128, T], bf16, tag="E", name=f"E{i}_{j}")
                    for sh in range((nchunks + 1) // 2):
                        c0, c1 = 2 * sh, min(nchunks, 2 * sh + 2)
                        S = psS.tile([128, 1024], f32, tag="S",
                                     name=f"S{i}_{j}_{sh}")
                        for c in range(c0, c1):
                            rh = 64 * (c % 2)
                            o = 512 * (c - c0)
                            n = min(512, W - 512 * c)
                            nc.tensor.matmul(
                                S[:, o:o + n],
                                lhsT=kRep[rh:rh + 64, half,
                                          128 * j:128 * (j + 1)],
                                rhs=qRep[rh:rh + 64, i, 512 * c:512 * c + n],
                                start=True, stop=True)
                        wv_ = min(1024, W - 1024 * sh)
                        # chunks fully inside the future region need no alibi
                        # bias (it cancels against the u multiplier exactly)
                        full_future = 1024 * (sh + 1) <= lo
                        nc.scalar.activation(
                            E[:, 1024 * sh:1024 * sh + wv_], S[:, :wv_],
                            AF.Exp,
                            bias=0.0 if full_future else biassb[:, i:i + 1],
                            scale=0.125)
                        if sh == 0 and mid is not None:
                            # the pending attn@v streams on the PE between
                            # this tile's two score chunks, giving the ACT
                            # time to drain exp(sh1) before the PE needs its
                            # psum slot back (removes the per-step lockstep)
                            mid()
                            mid = None
                    if mid is not None:
                        mid()
                    if W5 > W and any(_J_FIRST[i][tcn] == j
                                      for tcn in range(NCH)):
                        nc.vector.memset(E[:, W:W5], 0.0)
                    # diag multiplier min(exp(-a(t_in-127)), exp(a(127-s_in)))
                    if i not in DMIN:
                        DMIN[i] = work.tile([128, 128], bf16, tag="dmin",
                                            name=f"dm{i}")
                        nc.vector.tensor_scalar(DMIN[i], wrep[:, i, 0:128],
                                                usb[:, i:i + 1], None, MIN)
                    lo0 = (lo // 1024) * 1024  # u-mult on the partial chunk
                    if lo > lo0:
                        nc.vector.tensor_scalar(E[:, lo0:lo], E[:, lo0:lo],
                                                usb[:, i:i + 1], None, MUL)
                    nc.vector.tensor_tensor(E[:, lo:hi], E[:, lo:hi], DMIN[i],
                                            MUL)
                    if W > hi:
                        nc.vector.tensor_tensor(
                            E[:, hi:W], E[:, hi:W],
                            wrep[:, i, 128:128 + (W - hi)], MUL)
                    return E

                def emit_av(i, j, pa, E):
                    half = i % 2
                    W = _W128[i][j]
                    for tcn in range(_W512[i][j] // 512):
                        first = j == _J_FIRST[i][tcn]
                        # the initializing tile streams the full zero-padded
                        # 512 so the psum region is defined; later tiles
                        # stream exact widths
                        n = 512 if first else min(512, W - 512 * tcn)
                        nc.tensor.matmul(
                            pa[:, 512 * tcn:512 * tcn + n],
                            lhsT=v_sb[:, j, 65 * half:65 * half + 65],
                            rhs=E[:, 512 * tcn:512 * tcn + n],
                            start=first, stop=(j == ST - 1),
                            skip_group_check=True)

                def emit_epilogue(i, pa):
                    p, half = i // 2, i % 2
                    st65 = stp.tile([65, T], bf16, tag="st65", name=f"st{i}")
                    nc.vector.tensor_copy(st65, pa[0:65, :])
                    nc.sync.dma_start(out=outT[64 * half:64 * half + 64, p, :],
                                      in_=st65[0:64, :])
                    # denominator row -> [16,128] (partition-major reciprocal)
                    nc.sync.dma_start(out=drow_d[i:i + 1, :], in_=st65[64:65, :])
                    dsp = dpk.tile([16, 128], bf16, tag="dsp", name=f"dsp{i}")
                    nc.gpsimd.dma_start(
                        out=dsp, in_=drow_d[i].rearrange("(a b) -> a b", b=128))
                    rpf = dpk.tile([16, 128], f32, tag="rpf", name=f"rpf{i}")
                    nc.vector.reciprocal(rpf, dsp)
                    rp = dpk.tile([16, 128], bf16, tag="rp", name=f"rp{i}")
                    nc.vector.tensor_copy(rp, rpf)
                    nc.gpsimd.dma_start(
                        out=rrow_d[i].rearrange("(a b) -> a b", b=128), in_=rp)
                    if half == 0:
                        RR[p] = rrp.tile([128, T], bf16, tag="rr", name=f"rr{p}")
                    nc.gpsimd.dma_start(out=RR[p][64 * half:64 * half + 64, :],
                                        in_=bcast(rrow_d[i:i + 1, :], 64))
                    if half == 1:
                        nc.vector.tensor_tensor(outT[:, p, :], outT[:, p, :],
                                                RR[p], MUL)

                # ---- emission: software-pipelined attention stream ----
                # head 0 starts after only k(t<1024) + q(heads 0-1, t<1024);
                # the rest of the projections interleave into its j-loop
                k_proj_sh(0)
                q_half(0, 0)
                ilv = {
                    (0, 1): [lambda: v_half(0)],
                    (0, 2): [lambda: q_half(0, 1)],
                    (0, 3): [lambda: k_proj_sh(1)],
                    (0, 5): [lambda: v_half(1)],
                    (1, 4): [lambda: q_half(1, 0)],
                    (1, 8): [lambda: q_half(1, 1)],
                    (3, 4): [lambda: q_half(2, 0)],
                    (3, 8): [lambda: q_half(2, 1)],
                    (5, 4): [lambda: q_half(3, 0)],
                    (5, 8): [lambda: q_half(3, 1)],
                }
                pending = None   # (i, j, pa, E) awaiting attn@v emission
                epiq = []        # delayed epilogues: (steps_left, fn)
                for i in range(NH):
                    pa = psAp.tile([65, T], f32, tag="pa", name=f"pa{i}")
                    for j in range(ST):
                        for fn in ilv.get((i, j), ()):
                            fn()
                        # delayed epilogues fire BEFORE the attn@v flush: the
                        # epilogue of head i must be emitted before head i+1's
                        # first attn@v write reuses the psum accumulator
                        epiq = [(n - 1, fn) for n, fn in epiq]
                        for _, fn in [e for e in epiq if e[0] <= 0]:
                            fn()
                        epiq = [e for e in epiq if e[0] > 0]
                        mid = ((lambda pend=pending: emit_av(*pend))
                               if pending is not None else None)
                        E = emit_scores(i, j, mid)
                        pending = (i, j, pa, E)
                    if i < NH - 1:
                        # fire 2 pipeline steps into the next head so the
                        # epilogue DVE work queues behind the next head's
                        # multipliers
                        epiq.append((2, lambda pi=i, pp=pa:
                                     emit_epilogue(pi, pp)))
                pa_last = pending[2]
                emit_av(*pending)
                for _, fn in epiq:
                    fn()

                # ---- tail: the last head's epilogue is processed in 512-col
                # ---- quarters, each followed by its 4 output-projection
                # ---- tiles, so the PE streams the projection while the
                # ---- reciprocal chain of the next quarter is in flight
                oeng = [nc.sync, nc.scalar, nc.gpsimd]
                st65 = stp.tile([65, T], bf16, tag="st65", name="st7")
                rr16 = drow_d[NH - 1].rearrange("(a b) -> a b", b=128)
                rp16 = rrow_d[NH - 1].rearrange("(a b) -> a b", b=128)

                def pp_tile(tt):
                    pp = psS.tile([128, 1024], f32, tag="S", name=f"pp{tt}")
                    for ec in range(2):
                        for kt in range(4):
                            nc.tensor.matmul(
                                pp[:, 512 * ec:512 * (ec + 1)],
                                lhsT=outT[:, kt, 128 * tt:128 * (tt + 1)],
                                rhs=wp[:, kt, 512 * ec:512 * (ec + 1)],
                                start=(kt == 0), stop=(kt == 3))
                    osb = outp.tile([128, C], bf16, tag="osb", name=f"ob{tt}")
                    if tt % 2 == 0:
                        nc.scalar.copy(osb, pp)
                    else:
                        nc.vector.tensor_copy(osb, pp)
                    for ec in range(2):
                        oeng[(2 * tt + ec) % 3].dma_start(
                            out=out_d[128 * tt:128 * (tt + 1),
                                      512 * ec:512 * (ec + 1)],
                            in_=osb[:, 512 * ec:512 * (ec + 1)])

                # ones row on partition 64 (matches the denominator row's
                # partition) for the quarter-0 broadcast matmul
                ones1 = const.tile([128, 128], bf16)
                nc.vector.memset(ones1[64:65, :], 1.0)
                for q in range(NCH):
                    ql = slice(512 * q, 512 * (q + 1))
                    nc.vector.tensor_copy(st65[:, ql], pa_last[0:65, ql])
                    nc.sync.dma_start(out=outT[64:128, 3, ql],
                                      in_=st65[0:64, ql])
                    if q == 0:
                        # first quarter's chain is fully exposed: broadcast
                        # the denominator via a 1-contraction matmul (no DMA
                        # hops) and invert on the DVE
                        db = psS.tile([128, 512], f32, tag="S", name="db0")
                        nc.tensor.matmul(db, lhsT=ones1[64:65, :],
                                         rhs=st65[64:65, ql],
                                         start=True, stop=True)
                        rq = dpk.tile([128, 512], bf16, tag="rq", name="rq0")
                        with nc.allow_low_precision(reason="1/denom bf16"):
                            nc.vector.reciprocal(rq, db)
                        nc.vector.tensor_tensor(outT[0:64, 3, ql],
                                                outT[0:64, 3, ql],
                                                RR[3][0:64, ql], MUL)
                        nc.vector.tensor_tensor(outT[64:128, 3, ql],
                                                outT[64:128, 3, ql],
                                                rq[64:128, :], MUL)
                        for tt in range(4):
                            pp_tile(tt)
                        continue
                    nc.sync.dma_start(out=drow_d[NH - 1:NH, ql],
                                      in_=st65[64:65, ql])
                    dsp = dpk.tile([4, 128], bf16, tag="dsp", name=f"dspq{q}")
                    nc.gpsimd.dma_start(out=dsp, in_=rr16[4 * q:4 * q + 4, :])
                    rpf = dpk.tile([4, 128], f32, tag="rpf", name=f"rpfq{q}")
                    nc.vector.reciprocal(rpf, dsp)
                    rp = dpk.tile([4, 128], bf16, tag="rp", name=f"rpq{q}")
                    nc.vector.tensor_copy(rp, rpf)
                    nc.gpsimd.dma_start(out=rp16[4 * q:4 * q + 4, :], in_=rp)
                    nc.gpsimd.dma_start(
                        out=RR[3][64:128, ql],
                        in_=bcast(rrow_d[NH - 1:NH, ql], 64))
                    nc.vector.tensor_tensor(outT[:, 3, ql], outT[:, 3, ql],
                                            RR[3][:, ql], MUL)
                    for tt in range(4 * q, 4 * q + 4):
                        pp_tile(tt)

    _split_multiwait(nc, mybir)
    _NC_CACHE["nc"] = nc
    return nc


def _prep_core_inputs(x, Wq, Wkv, Wproj, b, g):
    import ml_dtypes
    bf = ml_dtypes.bfloat16
    heads = [_head_of_slot(i, g) for i in range(NH)]
    xT = np.ascontiguousarray(x[b].T).astype(bf)                      # [C, T]
    wq_cols = np.concatenate([Wq[64 * h:64 * (h + 1)] for h in heads], axis=0)
    wqT = np.ascontiguousarray(wq_cols.T).astype(bf)                  # [C, 512]
    wkT = np.ascontiguousarray(Wkv[128 * g:128 * (g + 1)].T).astype(bf)
    wvT = np.ascontiguousarray(Wkv[256 + 128 * g:256 + 128 * (g + 1)].T).astype(bf)
    cols = np.concatenate([np.arange(64 * h, 64 * (h + 1)) for h in heads])
    wpT = np.ascontiguousarray(Wproj[:, cols].T).astype(bf)           # [512, C]

    s_in = np.arange(128, dtype=np.float64)
    wrow = np.empty((NH, WREP_W), dtype=bf)
    u = np.empty((128, NH), dtype=np.float32)
    bias = np.empty((128, NH), dtype=np.float32)
    idx = np.arange(WREP_W, dtype=np.float64)
    for i, h in enumerate(heads):
        a = _a_of_head(h)
        wrow[i] = np.exp(-a * (idx - 127.0)).astype(np.float32)
        u[:, i] = np.exp(a * (127.0 - s_in)).astype(np.float32)
        bias[:, i] = (a * (s_in - 127.0)).astype(np.float32)
    return {"xT": xT, "wqT": wqT, "wkT": wkT, "wvT": wvT, "wpT": wpT,
            "wrow": wrow, "usb": u, "biassb": bias}


def kernel(x, Wq, Wkv, Wproj, bproj):
    from concourse.bass_utils import run_bass_kernel_spmd
    x = np.asarray(x, dtype=np.float32)
    Wq = np.asarray(Wq, dtype=np.float32)
    Wkv = np.asarray(Wkv, dtype=np.float32)
    Wproj = np.asarray(Wproj, dtype=np.float32)
    bproj = np.asarray(bproj, dtype=np.float32)

    nc = _build_nc()
    in_maps = [_prep_core_inputs(x, Wq, Wkv, Wproj, c // 2, c % 2)
               for c in range(8)]
    res = run_bass_kernel_spmd(nc, in_maps, core_ids=list(range(8)))
    out = np.zeros((B, T, C), dtype=np.float32)
    for c in range(8):
        out[c // 2] += np.asarray(res.results[c]["out"], dtype=np.float32)
    out += bproj[None, None, :]
    return out


# revision 50
# speedup vs baseline: 1.0025x; 1.0025x over previous
"""Bass/Tile TRN2 kernel for nn_DecoderGroupedQueryHeadAttentionAlibi.

Sharding (8 cores): core = (b, g) with b = core//2 in [0,4) (batch),
g = core%2 (head-half). Each core computes 8 of 16 query heads (those with
h%4 in {2g, 2g+1}) for its batch, plus the corresponding row-slice of the
output projection; the host sums the two half partials and adds bproj.

v3 changes over the baseline:
  - the j-loop is software-pipelined: attn@v for s-tile j is emitted after
    scores/exp/multipliers of s-tile j+1, so the PE streams the next tile's
    scores while the DVE finishes the current tile's row multipliers
    (removes a per-tile PE stall on the DVE).
  - x input DMA is split into t-halves so the first projections start
    after half the input has landed.
  - psum attn accumulator is split into two [65,1024] tiles so the next
    head's attn@v can start after half the epilogue copy.
  - per-head epilogue: direct [1,T] reciprocal + one DRAM-broadcast hop
    (replaces a 4-hop DMA round-trip chain).
  - output projection reuses the score PSUM pool (no pool-transition
    barrier), writes bf16 (summed on host in fp32), pipelines per t-tile,
    and spreads the output DMA over 3 queues.

Per-core device program (layout A, scoresT = [s_partitions, t_free]):
  per (head, s-tile): scoresT psum [128,1024] tiles -> ACT exp (alibi bias
  folded into the per-partition activation bias) -> DVE multiplier applied
  per region (past/diag/future) -> attn@v psum accumulation where row 64
  (a ones column in v) is the softmax denominator.

The alibi bias of this module is min(a_h*(s-t), 0) (tril overwrites the
causal mask in the torch reference, so future tokens are attended with bias
0), hence P = exp(score/8) * min(exp(a*(s-t)), 1), which factors into a
per-partition ACT bias exp(a*(s_in-127)) and a distance-only (Toeplitz) row
multiplier exp(-a*delta). Score columns with t - s > margin/a are dropped
(banded): the dropped softmax mass is < e^-margin of the kept band mass.
"""

import math
import numpy as np

# ---- problem constants (hardcoded; kernel.py must be self-contained) ----
B, T, C = 4, 2048, 1024
N_HEAD, N_KV_HEAD, HEAD_DIM = 16, 4, 64
NH = 8            # heads per core
ST = T // 128     # 16 s-tiles
NCH = T // 512    # 4 t-chunks
KCT = C // 128    # 8 contraction tiles of 128
WREP_W = 2048     # Toeplitz table width: index = t - 128*j is always < 2048
MARGIN = 4.5      # exp(-4.5): adds ~3e-3 relmax (validated), total ~9e-3 < 2e-2

_START = 2.0 ** (-2.0 ** (-(math.log2(N_HEAD) - 3.0)))  # 0.7071...


def _head_of_slot(i: int, g: int) -> int:
    return 4 * (i // 2) + 2 * g + (i % 2)


def _a_of_head(h: int) -> float:
    return (_START ** (h + 1)) / math.sqrt(HEAD_DIM)


# Loop bounds must be identical on every core (SPMD): use the widest cutoff
# over g for each head slot (g=1 heads have smaller slopes -> wider bands).
_CUTOFF = [MARGIN / min(_a_of_head(_head_of_slot(i, 0)),
                        _a_of_head(_head_of_slot(i, 1)))
           for i in range(NH)]
_R128 = [min(T, int(math.ceil(c / 128.0)) * 128) for c in _CUTOFF]
# 128-granular computed width per (head slot, s-tile): scores/exp/multiplies
_W128 = [[min(T, 128 * (j + 1) + _R128[i]) for j in range(ST)]
         for i in range(NH)]
# 512-granular width for the attn@v accumulation (E tail is zeroed)
_W512 = [[((w + 511) // 512) * 512 for w in row] for row in _W128]
_NEFF = [[w // 512 for w in row] for row in _W512]
_J_FIRST = [[min(j for j in range(ST) if _NEFF[i][j] > tcn)
             for tcn in range(NCH)] for i in range(NH)]

_NC_CACHE = {}


def _split_multiwait(nc, mybir, max_waits=1):
    """walrus in this env encodes at most one sync-wait per instruction;
    split extras onto same-engine NoOps emitted just before."""
    for f in nc.m.functions:
        for bb in f.blocks:
            new = []
            for ins in bb.instructions:
                si = ins.sync_info
                conds = list(si.on_wait) if si is not None else []
                if len(conds) > max_waits:
                    for cond in conds[:-max_waits]:
                        n = mybir.InstNoOp(
                            name=nc.get_next_instruction_name(), ins=[], outs=[])
                        n.engine = ins.engine
                        n.sync_info = mybir.SyncInfo(on_wait=[cond], on_update=[])
                        new.append(n)
                    si.on_wait = conds[-max_waits:]
                new.append(ins)
            bb.instructions = new


def _build_nc():
    if "nc" in _NC_CACHE:
        return _NC_CACHE["nc"]
    import concourse.bass as bass
    import concourse.tile as tile
    from concourse import mybir

    f32 = mybir.dt.float32
    bf16 = mybir.dt.bfloat16
    AF = mybir.ActivationFunctionType
    MUL = mybir.AluOpType.mult
    MIN = mybir.AluOpType.min

    nc = bass.Bass()

    xT_d = nc.dram_tensor("xT", [C, T], bf16, kind="ExternalInput")
    wq_d = nc.dram_tensor("wqT", [C, NH * 64], bf16, kind="ExternalInput")
    wk_d = nc.dram_tensor("wkT", [C, 128], bf16, kind="ExternalInput")
    wv_d = nc.dram_tensor("wvT", [C, 128], bf16, kind="ExternalInput")
    wp_d = nc.dram_tensor("wpT", [NH * 64, C], bf16, kind="ExternalInput")
    wrow_d = nc.dram_tensor("wrow", [NH, WREP_W], bf16, kind="ExternalInput")
    u_d = nc.dram_tensor("usb", [128, NH], f32, kind="ExternalInput")
    bias_d = nc.dram_tensor("biassb", [128, NH], f32, kind="ExternalInput")
    out_d = nc.dram_tensor("out", [T, C], bf16, kind="ExternalOutput")

    xT_r = xT_d.rearrange("(k p) t -> p k t", p=128)
    wq_r = wq_d.rearrange("(k p) e -> p k e", p=128)

    def bcast(src_row, parts):
        # [1, W] DRAM row -> [parts, W] stride-0 partition broadcast source
        return bass.AP(tensor=src_row.tensor, offset=src_row.offset,
                       ap=[[0, parts]] + list(src_row.ap)[1:])

    with tile.TileContext(nc) as tc:
        with (
            tc.tile_pool(name="const", bufs=1) as const,
            tc.tile_pool(name="work", bufs=3) as work,
            tc.tile_pool(name="ebuf", bufs=5) as ebufp,
            tc.tile_pool(name="stp", bufs=2) as stp,
            tc.tile_pool(name="rrp", bufs=2) as rrp,
            tc.tile_pool(name="dpk", bufs=4) as dpk,
            tc.tile_pool(name="outp", bufs=4) as outp,
            tc.tile_pool(name="dramd", bufs=1, space="DRAM") as dramd,
        ):
            # ---- persistent tiles ----
            kRep = const.tile([128, 2, T], bf16)     # kv on both halves
            v_sb = const.tile([128, ST, 130], bf16)  # [s, j, (v_kv0|1|v_kv1|1)]
            qRep = const.tile([128, NH, T], bf16)    # head i on both halves
            outT = const.tile([128, 4, T], bf16)     # [(2 heads d), pair, t]
            wrep = const.tile([128, NH, WREP_W], bf16)
            wp = const.tile([128, 4, C], bf16)
            usb = const.tile([128, NH], f32)
            biassb = const.tile([128, NH], f32)
            xT = const.tile([128, KCT, T], bf16)
            wq = const.tile([128, KCT, NH * 64], bf16)
            wk = const.tile([128, KCT, 128], bf16)
            wv = const.tile([128, KCT, 128], bf16)
            warm = const.tile([128, 1], f32)
            wsink = const.tile([128, 1], f32)
            drow_d = dramd.tile([NH, T], bf16)
            rrow_d = dramd.tile([NH, T], bf16)

            # ---- ACT exp-table preload (runs during the DMA ramp) ----
            nc.vector.memset(warm, 0.0)
            nc.scalar.activation(wsink, warm, AF.Exp, scale=1.0)

            # ---- input DMAs; x lands in t-order (first 512 cols of every
            # ---- contraction chunk first) so the projections start early
            for sc in range(2):
                for kc in range(KCT):
                    eng = nc.sync if kc % 2 == 0 else nc.scalar
                    eng.dma_start(out=xT[:, kc, 512 * sc:512 * (sc + 1)],
                                  in_=xT_r[:, kc, 512 * sc:512 * (sc + 1)])
            for kc in range(KCT):
                eng = nc.sync if kc % 2 == 0 else nc.scalar
                eng.dma_start(out=xT[:, kc, 1024:2048],
                              in_=xT_r[:, kc, 1024:2048])
            nc.gpsimd.dma_start(out=wk, in_=wk_d.rearrange("(k p) e -> p k e", p=128))
            nc.gpsimd.dma_start(out=usb, in_=u_d[:])
            nc.gpsimd.dma_start(out=biassb, in_=bias_d[:])
            wrow_r = [wrow_d[i:i + 1, :] for i in range(NH)]

            def wrep_bc(i):
                nc.gpsimd.dma_start(out=wrep[:, i, :], in_=bcast(wrow_r[i], 128))

            wrep_bc(0)
            wrep_bc(1)
            for kc in range(KCT):
                nc.gpsimd.dma_start(out=wq[:, kc, :], in_=wq_r[:, kc, :])
            nc.gpsimd.dma_start(out=wv, in_=wv_d.rearrange("(k p) e -> p k e", p=128))
            for i in range(2, NH):
                wrep_bc(i)
            nc.gpsimd.dma_start(out=wp, in_=wp_d.rearrange("(k p) e -> p k e", p=128))

            with (
                tc.tile_pool(name="psS", bufs=2, space="PSUM") as psS,
                tc.tile_pool(name="psA", bufs=1, space="PSUM") as psAp,
            ):
                def _copy(eng, out, in_):
                    if eng is nc.scalar:
                        eng.copy(out, in_)
                    else:
                        eng.tensor_copy(out, in_)

                # ---- projection emitters (share the psS psum pool) ----
                def k_proj_sh(sh):
                    ceng = nc.vector
                    ps = psS.tile([128, 1024], f32, tag="S", name=f"kp{sh}")
                    for sub in range(2):
                        sc = 2 * sh + sub
                        for kc in range(KCT):
                            nc.tensor.matmul(
                                ps[:, 512 * sub:512 * (sub + 1)],
                                lhsT=wk[:, kc, :],
                                rhs=xT[:, kc, 512 * sc:512 * (sc + 1)],
                                start=(kc == 0), stop=(kc == KCT - 1))
                    sl = slice(1024 * sh, 1024 * (sh + 1))
                    _copy(ceng, kRep[0:64, 0, sl], ps[0:64, :])
                    _copy(ceng, kRep[64:128, 1, sl], ps[64:128, :])
                    nc.sync.dma_start(out=kRep[64:128, 0, sl],
                                      in_=kRep[0:64, 0, sl])
                    nc.sync.dma_start(out=kRep[0:64, 1, sl],
                                      in_=kRep[64:128, 1, sl])

                def q_half(p, h):
                    ceng = nc.vector
                    ps = psS.tile([128, 1024], f32, tag="S", name=f"qp{p}{h}")
                    for sub in range(2):
                        tcn = 2 * h + sub
                        for kc in range(KCT):
                            nc.tensor.matmul(
                                ps[:, 512 * sub:512 * (sub + 1)],
                                lhsT=wq[:, kc, 128 * p:128 * (p + 1)],
                                rhs=xT[:, kc, 512 * tcn:512 * (tcn + 1)],
                                start=(kc == 0), stop=(kc == KCT - 1))
                    sl = slice(1024 * h, 1024 * (h + 1))
                    _copy(ceng, qRep[0:64, 2 * p, sl], ps[0:64, :])
                    _copy(ceng, qRep[64:128, 2 * p + 1, sl], ps[64:128, :])
                    nc.sync.dma_start(out=qRep[64:128, 2 * p, sl],
                                      in_=qRep[0:64, 2 * p, sl])
                    nc.sync.dma_start(out=qRep[0:64, 2 * p + 1, sl],
                                      in_=qRep[64:128, 2 * p + 1, sl])

                def v_half(h):
                    ceng = nc.vector
                    ps = psS.tile([128, 1024], f32, tag="S", name=f"vh{h}")
                    for b in range(8):
                        st = 8 * h + b
                        for kc in range(KCT):
                            nc.tensor.matmul(
                                ps[:, 128 * b:128 * (b + 1)],
                                lhsT=xT[:, kc, 128 * st:128 * (st + 1)],
                                rhs=wv[:, kc, :],
                                start=(kc == 0), stop=(kc == KCT - 1))
                    ps3 = ps.rearrange("p (s d) -> p s d", d=128)
                    sl = slice(8 * h, 8 * (h + 1))
                    _copy(ceng, v_sb[:, sl, 0:64], ps3[:, :, 0:64])
                    _copy(ceng, v_sb[:, sl, 65:129], ps3[:, :, 64:128])
                    nc.vector.memset(v_sb[:, sl, 64], 1.0)
                    nc.vector.memset(v_sb[:, sl, 129], 1.0)

                RR = {}
                DMIN = {}

                def emit_scores(i, j, mid=None):
                    p, half = i // 2, i % 2
                    W, W5 = _W128[i][j], _W512[i][j]
                    lo = 128 * j         # t < lo : future region
                    hi = 128 * (j + 1)   # t >= hi: past region (Toeplitz)
                    nchunks = (W + 511) // 512
                    E = ebufp.tile([128, T], bf16, tag="E", name=f"E{i}_{j}")
                    for sh in range((nchunks + 1) // 2):
                        c0, c1 = 2 * sh, min(nchunks, 2 * sh + 2)
                        S = psS.tile([128, 1024], f32, tag="S",
                                     name=f"S{i}_{j}_{sh}")
                        for c in range(c0, c1):
                            rh = 64 * (c % 2)
                            o = 512 * (c - c0)
                            n = min(512, W - 512 * c)
                            nc.tensor.matmul(
                                S[:, o:o + n],
                                lhsT=kRep[rh:rh + 64, half,
                                          128 * j:128 * (j + 1)],
                                rhs=qRep[rh:rh + 64, i, 512 * c:512 * c + n],
                                start=True, stop=True)
                        wv_ = min(1024, W - 1024 * sh)
                        # chunks fully inside the future region need no alibi
                        # bias (it cancels against the u multiplier exactly)
                        full_future = 1024 * (sh + 1) <= lo
                        nc.scalar.activation(
                            E[:, 1024 * sh:1024 * sh + wv_], S[:, :wv_],
                            AF.Exp,
                            bias=0.0 if full_future else biassb[:, i:i + 1],
                            scale=0.125)
                        if sh == 0 and mid is not None:
                            # the pending attn@v streams on the PE between
                            # this tile's two score chunks, giving the ACT
                            # time to drain exp(sh1) before the PE needs its
                            # psum slot back (removes the per-step lockstep)
                            mid()
                            mid = None
                    if mid is not None:
                        mid()
                    if W5 > W and any(_J_FIRST[i][tcn] == j
                                      for tcn in range(NCH)):
                        nc.vector.memset(E[:, W:W5], 0.0)
                    # diag multiplier min(exp(-a(t_in-127)), exp(a(127-s_in)))
                    if i not in DMIN:
                        DMIN[i] = work.tile([128, 128], bf16, tag="dmin",
                                            name=f"dm{i}")
                        nc.vector.tensor_scalar(DMIN[i], wrep[:, i, 0:128],
                                                usb[:, i:i + 1], None, MIN)
                    lo0 = (lo // 1024) * 1024  # u-mult on the partial chunk
                    if lo > lo0:
                        nc.vector.tensor_scalar(E[:, lo0:lo], E[:, lo0:lo],
                                                usb[:, i:i + 1], None, MUL)
                    nc.vector.tensor_tensor(E[:, lo:hi], E[:, lo:hi], DMIN[i],
                                            MUL)
                    if W > hi:
                        nc.vector.tensor_tensor(
                            E[:, hi:W], E[:, hi:W],
                            wrep[:, i, 128:128 + (W - hi)], MUL)
                    return E

                def emit_av(i, j, pa, E):
                    half = i % 2
                    W = _W128[i][j]
                    for tcn in range(_W512[i][j] // 512):
                        first = j == _J_FIRST[i][tcn]
                        # the initializing tile streams the full zero-padded
                        # 512 so the psum region is defined; later tiles
                        # stream exact widths
                        n = 512 if first else min(512, W - 512 * tcn)
                        nc.tensor.matmul(
                            pa[:, 512 * tcn:512 * tcn + n],
                            lhsT=v_sb[:, j, 65 * half:65 * half + 65],
                            rhs=E[:, 512 * tcn:512 * tcn + n],
                            start=first, stop=(j == ST - 1),
                            skip_group_check=True)

                def emit_epilogue(i, pa):
                    p, half = i // 2, i % 2
                    st65 = stp.tile([65, T], bf16, tag="st65", name=f"st{i}")
                    nc.vector.tensor_copy(st65, pa[0:65, :])
                    nc.sync.dma_start(out=outT[64 * half:64 * half + 64, p, :],
                                      in_=st65[0:64, :])
                    # denominator row -> [16,128] (partition-major reciprocal)
                    nc.sync.dma_start(out=drow_d[i:i + 1, :], in_=st65[64:65, :])
                    dsp = dpk.tile([16, 128], bf16, tag="dsp", name=f"dsp{i}")
                    nc.gpsimd.dma_start(
                        out=dsp, in_=drow_d[i].rearrange("(a b) -> a b", b=128))
                    rpf = dpk.tile([16, 128], f32, tag="rpf", name=f"rpf{i}")
                    nc.vector.reciprocal(rpf, dsp)
                    rp = dpk.tile([16, 128], bf16, tag="rp", name=f"rp{i}")
                    nc.vector.tensor_copy(rp, rpf)
                    nc.gpsimd.dma_start(
                        out=rrow_d[i].rearrange("(a b) -> a b", b=128), in_=rp)
                    if half == 0:
                        RR[p] = rrp.tile([128, T], bf16, tag="rr", name=f"rr{p}")
                    nc.gpsimd.dma_start(out=RR[p][64 * half:64 * half + 64, :],
                                        in_=bcast(rrow_d[i:i + 1, :], 64))
                    if half == 1:
                        nc.vector.tensor_tensor(outT[:, p, :], outT[:, p, :],
                                                RR[p], MUL)

                # ---- emission: software-pipelined attention stream ----
                # head 0 starts after only k(t<1024) + q(heads 0-1, t<1024);
                # the rest of the projections interleave into its j-loop
                k_proj_sh(0)
                q_half(0, 0)
                ilv = {
                    (0, 1): [lambda: v_half(0)],
                    (0, 2): [lambda: q_half(0, 1)],
                    (0, 3): [lambda: k_proj_sh(1)],
                    (0, 5): [lambda: v_half(1)],
                    (1, 4): [lambda: q_half(1, 0)],
                    (1, 8): [lambda: q_half(1, 1)],
                    (3, 4): [lambda: q_half(2, 0)],
                    (3, 8): [lambda: q_half(2, 1)],
                    (5, 4): [lambda: q_half(3, 0)],
                    (5, 8): [lambda: q_half(3, 1)],
                }
                pending = None   # (i, j, pa, E) awaiting attn@v emission
                epiq = []        # delayed epilogues: (steps_left, fn)
                for i in range(NH):
                    pa = psAp.tile([65, T], f32, tag="pa", name=f"pa{i}")
                    for j in range(ST):
                        for fn in ilv.get((i, j), ()):
                            fn()
                        # delayed epilogues fire BEFORE the attn@v flush: the
                        # epilogue of head i must be emitted before head i+1's
                        # first attn@v write reuses the psum accumulator
                        epiq = [(n - 1, fn) for n, fn in epiq]
                        for _, fn in [e for e in epiq if e[0] <= 0]:
                            fn()
                        epiq = [e for e in epiq if e[0] > 0]
                        mid = ((lambda pend=pending: emit_av(*pend))
                               if pending is not None else None)
                        E = emit_scores(i, j, mid)
                        pending = (i, j, pa, E)
                    if i < NH - 1:
                        # fire 2 pipeline steps into the next head so the
                        # epilogue DVE work queues behind the next head's
                        # multipliers
                        epiq.append((2, lambda pi=i, pp=pa:
                                     emit_epilogue(pi, pp)))
                pa_last = pending[2]
                emit_av(*pending)
                for _, fn in epiq:
                    fn()

                # ---- tail: the last head's epilogue is processed in 512-col
                # ---- quarters, each followed by its 4 output-projection
                # ---- tiles, so the PE streams the projection while the
                # ---- reciprocal chain of the next quarter is in flight
                oeng = [nc.sync, nc.scalar, nc.gpsimd]
                st65 = stp.tile([65, T], bf16, tag="st65", name="st7")
                rr16 = drow_d[NH - 1].rearrange("(a b) -> a b", b=128)
                rp16 = rrow_d[NH - 1].rearrange("(a b) -> a b", b=128)

                def pp_tile(tt):
                    pp = psS.tile([128, 1024], f32, tag="S", name=f"pp{tt}")
                    for ec in range(2):
                        for kt in range(4):
                            nc.tensor.matmul(
                                pp[:, 512 * ec:512 * (ec + 1)],
                                lhsT=outT[:, kt, 128 * tt:128 * (tt + 1)],
                                rhs=wp[:, kt, 512 * ec:512 * (ec + 1)],
                                start=(kt == 0), stop=(kt == 3))
                    osb = outp.tile([128, C], bf16, tag="osb", name=f"ob{tt}")
                    # halves copy on ACT and DVE in parallel (both idle at
                    # the tail) so each tile's DMA can start sooner
                    nc.scalar.copy(osb[:, 0:512], pp[:, 0:512])
                    nc.vector.tensor_copy(osb[:, 512:1024], pp[:, 512:1024])
                    for ec in range(2):
                        oeng[(2 * tt + ec) % 3].dma_start(
                            out=out_d[128 * tt:128 * (tt + 1),
                                      512 * ec:512 * (ec + 1)],
                            in_=osb[:, 512 * ec:512 * (ec + 1)])

                for q in range(NCH):
                    ql = slice(512 * q, 512 * (q + 1))
                    nc.vector.tensor_copy(st65[:, ql], pa_last[0:65, ql])
                    nc.sync.dma_start(out=outT[64:128, 3, ql],
                                      in_=st65[0:64, ql])
                    nc.sync.dma_start(out=drow_d[NH - 1:NH, ql],
                                      in_=st65[64:65, ql])
                    dsp = dpk.tile([4, 128], bf16, tag="dsp", name=f"dspq{q}")
                    nc.gpsimd.dma_start(out=dsp, in_=rr16[4 * q:4 * q + 4, :])
                    rpf = dpk.tile([4, 128], f32, tag="rpf", name=f"rpfq{q}")
                    nc.vector.reciprocal(rpf, dsp)
                    rp = dpk.tile([4, 128], bf16, tag="rp", name=f"rpq{q}")
                    nc.vector.tensor_copy(rp, rpf)
                    nc.gpsimd.dma_start(out=rp16[4 * q:4 * q + 4, :], in_=rp)
                    nc.gpsimd.dma_start(
                        out=RR[3][64:128, ql],
                        in_=bcast(rrow_d[NH - 1:NH, ql], 64))
                    nc.vector.tensor_tensor(outT[:, 3, ql], outT[:, 3, ql],
                                            RR[3][:, ql], MUL)
                    for tt in range(4 * q, 4 * q + 4):
                        pp_tile(tt)

    _split_multiwait(nc, mybir)
    _NC_CACHE["nc"] = nc
    return nc


def _prep_core_inputs(x, Wq, Wkv, Wproj, b, g):
    import ml_dtypes
    bf = ml_dtypes.bfloat16
    heads = [_head_of_slot(i, g) for i in range(NH)]
    xT = np.ascontiguousarray(x[b].T).astype(bf)                      # [C, T]
    wq_cols = np.concatenate([Wq[64 * h:64 * (h + 1)] for h in heads], axis=0)
    wqT = np.ascontiguousarray(wq_cols.T).astype(bf)                  # [C, 512]
    wkT = np.ascontiguousarray(Wkv[128 * g:128 * (g + 1)].T).astype(bf)
    wvT = np.ascontiguousarray(Wkv[256 + 128 * g:256 + 128 * (g + 1)].T).astype(bf)
    cols = np.concatenate([np.arange(64 * h, 64 * (h + 1)) for h in heads])
    wpT = np.ascontiguousarray(Wproj[:, cols].T).astype(bf)           # [512, C]

    s_in = np.arange(128, dtype=np.float64)
    wrow = np.empty((NH, WREP_W), dtype=bf)
    u = np.empty((128, NH), dtype=np.float32)
    bias = np.empty((128, NH), dtype=np.float32)
    idx = np.arange(WREP_W, dtype=np.float64)
    for i, h in enumerate(heads):
        a = _a_of_head(h)
        wrow[i] = np.exp(-a * (idx - 127.0)).astype(np.float32)
        u[:, i] = np.exp(a * (127.0 - s_in)).astype(np.float32)
        bias[:, i] = (a * (s_in - 127.0)).astype(np.float32)
    return {"xT": xT, "wqT": wqT, "wkT": wkT, "wvT": wvT, "wpT": wpT,
            "wrow": wrow, "usb": u, "biassb": bias}


def kernel(x, Wq, Wkv, Wproj, bproj):
    from concourse.bass_utils import run_bass_kernel_spmd
    x = np.asarray(x, dtype=np.float32)
    Wq = np.asarray(Wq, dtype=np.float32)
    Wkv = np.asarray(Wkv, dtype=np.float32)
    Wproj = np.asarray(Wproj, dtype=np.float32)
    bproj = np.asarray(bproj, dtype=np.float32)

    nc = _build_nc()
    in_maps = [_prep_core_inputs(x, Wq, Wkv, Wproj, c // 2, c % 2)
               for c in range(8)]
    res = run_bass_kernel_spmd(nc, in_maps, core_ids=list(range(8)))
    out = np.zeros((B, T, C), dtype=np.float32)
    for c in range(8):
        out[c // 2] += np.asarray(res.results[c]["out"], dtype=np.float32)
    out += bproj[None, None, :]
    return out


# revision 52
# speedup vs baseline: 1.0127x; 1.0102x over previous
"""Bass/Tile TRN2 kernel for nn_DecoderGroupedQueryHeadAttentionAlibi.

Sharding (8 cores): core = (b, g) with b = core//2 in [0,4) (batch),
g = core%2 (head-half). Each core computes 8 of 16 query heads (those with
h%4 in {2g, 2g+1}) for its batch, plus the corresponding row-slice of the
output projection; the host sums the two half partials and adds bproj.

v3 changes over the baseline:
  - the j-loop is software-pipelined: attn@v for s-tile j is emitted after
    scores/exp/multipliers of s-tile j+1, so the PE streams the next tile's
    scores while the DVE finishes the current tile's row multipliers
    (removes a per-tile PE stall on the DVE).
  - x input DMA is split into t-halves so the first projections start
    after half the input has landed.
  - psum attn accumulator is split into two [65,1024] tiles so the next
    head's attn@v can start after half the epilogue copy.
  - per-head epilogue: direct [1,T] reciprocal + one DRAM-broadcast hop
    (replaces a 4-hop DMA round-trip chain).
  - output projection reuses the score PSUM pool (no pool-transition
    barrier), writes bf16 (summed on host in fp32), pipelines per t-tile,
    and spreads the output DMA over 3 queues.

Per-core device program (layout A, scoresT = [s_partitions, t_free]):
  per (head, s-tile): scoresT psum [128,1024] tiles -> ACT exp (alibi bias
  folded into the per-partition activation bias) -> DVE multiplier applied
  per region (past/diag/future) -> attn@v psum accumulation where row 64
  (a ones column in v) is the softmax denominator.

The alibi bias of this module is min(a_h*(s-t), 0) (tril overwrites the
causal mask in the torch reference, so future tokens are attended with bias
0), hence P = exp(score/8) * min(exp(a*(s-t)), 1), which factors into a
per-partition ACT bias exp(a*(s_in-127)) and a distance-only (Toeplitz) row
multiplier exp(-a*delta). Score columns with t - s > margin/a are dropped
(banded): the dropped softmax mass is < e^-margin of the kept band mass.
"""

import math
import numpy as np

# ---- problem constants (hardcoded; kernel.py must be self-contained) ----
B, T, C = 4, 2048, 1024
N_HEAD, N_KV_HEAD, HEAD_DIM = 16, 4, 64
NH = 8            # heads per core
ST = T // 128     # 16 s-tiles
NCH = T // 512    # 4 t-chunks
KCT = C // 128    # 8 contraction tiles of 128
WREP_W = 2048     # Toeplitz table width: index = t - 128*j is always < 2048
MARGIN = 4.5      # exp(-4.5): adds ~3e-3 relmax (validated), total ~9e-3 < 2e-2

_START = 2.0 ** (-2.0 ** (-(math.log2(N_HEAD) - 3.0)))  # 0.7071...


def _head_of_slot(i: int, g: int) -> int:
    return 4 * (i // 2) + 2 * g + (i % 2)


def _a_of_head(h: int) -> float:
    return (_START ** (h + 1)) / math.sqrt(HEAD_DIM)


# Loop bounds must be identical on every core (SPMD): use the widest cutoff
# over g for each head slot (g=1 heads have smaller slopes -> wider bands).
_CUTOFF = [MARGIN / min(_a_of_head(_head_of_slot(i, 0)),
                        _a_of_head(_head_of_slot(i, 1)))
           for i in range(NH)]
_R128 = [min(T, int(math.ceil(c / 128.0)) * 128) for c in _CUTOFF]
# 128-granular computed width per (head slot, s-tile): scores/exp/multiplies
_W128 = [[min(T, 128 * (j + 1) + _R128[i]) for j in range(ST)]
         for i in range(NH)]
# 512-granular width for the attn@v accumulation (E tail is zeroed)
_W512 = [[((w + 511) // 512) * 512 for w in row] for row in _W128]
_NEFF = [[w // 512 for w in row] for row in _W512]
_J_FIRST = [[min(j for j in range(ST) if _NEFF[i][j] > tcn)
             for tcn in range(NCH)] for i in range(NH)]

_NC_CACHE = {}


def _split_multiwait(nc, mybir, max_waits=1):
    """walrus in this env encodes at most one sync-wait per instruction;
    split extras onto same-engine NoOps emitted just before."""
    for f in nc.m.functions:
        for bb in f.blocks:
            new = []
            for ins in bb.instructions:
                si = ins.sync_info
                conds = list(si.on_wait) if si is not None else []
                if len(conds) > max_waits:
                    for cond in conds[:-max_waits]:
                        n = mybir.InstNoOp(
                            name=nc.get_next_instruction_name(), ins=[], outs=[])
                        n.engine = ins.engine
                        n.sync_info = mybir.SyncInfo(on_wait=[cond], on_update=[])
                        new.append(n)
                    si.on_wait = conds[-max_waits:]
                new.append(ins)
            bb.instructions = new


def _build_nc():
    if "nc" in _NC_CACHE:
        return _NC_CACHE["nc"]
    import concourse.bass as bass
    import concourse.tile as tile
    from concourse import mybir

    f32 = mybir.dt.float32
    bf16 = mybir.dt.bfloat16
    AF = mybir.ActivationFunctionType
    MUL = mybir.AluOpType.mult
    MIN = mybir.AluOpType.min

    nc = bass.Bass()

    xT_d = nc.dram_tensor("xT", [C, T], bf16, kind="ExternalInput")
    wq_d = nc.dram_tensor("wqT", [C, NH * 64], bf16, kind="ExternalInput")
    wk_d = nc.dram_tensor("wkT", [C, 128], bf16, kind="ExternalInput")
    wv_d = nc.dram_tensor("wvT", [C, 128], bf16, kind="ExternalInput")
    wp_d = nc.dram_tensor("wpT", [NH * 64, C], bf16, kind="ExternalInput")
    wrow_d = nc.dram_tensor("wrow", [NH, WREP_W], bf16, kind="ExternalInput")
    u_d = nc.dram_tensor("usb", [128, NH], f32, kind="ExternalInput")
    bias_d = nc.dram_tensor("biassb", [128, NH], f32, kind="ExternalInput")
    out_d = nc.dram_tensor("out", [T, C], bf16, kind="ExternalOutput")

    xT_r = xT_d.rearrange("(k p) t -> p k t", p=128)
    wq_r = wq_d.rearrange("(k p) e -> p k e", p=128)

    def bcast(src_row, parts):
        # [1, W] DRAM row -> [parts, W] stride-0 partition broadcast source
        return bass.AP(tensor=src_row.tensor, offset=src_row.offset,
                       ap=[[0, parts]] + list(src_row.ap)[1:])

    with tile.TileContext(nc) as tc:
        with (
            tc.tile_pool(name="const", bufs=1) as const,
            tc.tile_pool(name="work", bufs=3) as work,
            tc.tile_pool(name="ebuf", bufs=4) as ebufp,
            tc.tile_pool(name="stp", bufs=2) as stp,
            tc.tile_pool(name="rrp", bufs=2) as rrp,
            tc.tile_pool(name="dpk", bufs=4) as dpk,
            tc.tile_pool(name="outp", bufs=4) as outp,
            tc.tile_pool(name="dramd", bufs=1, space="DRAM") as dramd,
        ):
            # ---- persistent tiles ----
            kRep = const.tile([128, 2, T], bf16)     # kv on both halves
            v_sb = const.tile([128, ST, 130], bf16)  # [s, j, (v_kv0|1|v_kv1|1)]
            qRep = const.tile([128, NH, T], bf16)    # head i on both halves
            outT = const.tile([128, 4, T], bf16)     # [(2 heads d), pair, t]
            wrep = const.tile([128, NH, WREP_W], bf16)
            wp = const.tile([128, 4, C], bf16)
            usb = const.tile([128, NH], f32)
            biassb = const.tile([128, NH], f32)
            xT = const.tile([128, KCT, T], bf16)
            wq = const.tile([128, KCT, NH * 64], bf16)
            wk = const.tile([128, KCT, 128], bf16)
            wv = const.tile([128, KCT, 128], bf16)
            warm = const.tile([128, 1], f32)
            wsink = const.tile([128, 1], f32)
            drow_d = dramd.tile([NH, T], bf16)
            rrow_d = dramd.tile([NH, T], bf16)

            # ---- ACT exp-table preload (runs during the DMA ramp) ----
            nc.vector.memset(warm, 0.0)
            nc.scalar.activation(wsink, warm, AF.Exp, scale=1.0)

            # ---- input DMAs; x lands in t-order (first 512 cols of every
            # ---- contraction chunk first) so the projections start early
            for sc in range(2):
                for kc in range(KCT):
                    eng = nc.sync if kc % 2 == 0 else nc.scalar
                    eng.dma_start(out=xT[:, kc, 512 * sc:512 * (sc + 1)],
                                  in_=xT_r[:, kc, 512 * sc:512 * (sc + 1)])
            for kc in range(KCT):
                eng = nc.sync if kc % 2 == 0 else nc.scalar
                eng.dma_start(out=xT[:, kc, 1024:2048],
                              in_=xT_r[:, kc, 1024:2048])
            nc.gpsimd.dma_start(out=wk, in_=wk_d.rearrange("(k p) e -> p k e", p=128))
            nc.gpsimd.dma_start(out=usb, in_=u_d[:])
            nc.gpsimd.dma_start(out=biassb, in_=bias_d[:])
            wrow_r = [wrow_d[i:i + 1, :] for i in range(NH)]

            def wrep_bc(i):
                nc.gpsimd.dma_start(out=wrep[:, i, :], in_=bcast(wrow_r[i], 128))

            wrep_bc(0)
            wrep_bc(1)
            for kc in range(KCT):
                nc.gpsimd.dma_start(out=wq[:, kc, :], in_=wq_r[:, kc, :])
            nc.gpsimd.dma_start(out=wv, in_=wv_d.rearrange("(k p) e -> p k e", p=128))
            for i in range(2, NH):
                wrep_bc(i)
            nc.gpsimd.dma_start(out=wp, in_=wp_d.rearrange("(k p) e -> p k e", p=128))

            with (
                tc.tile_pool(name="psS", bufs=2, space="PSUM") as psS,
                tc.tile_pool(name="psA", bufs=1, space="PSUM") as psAp,
            ):
                def _copy(eng, out, in_):
                    if eng is nc.scalar:
                        eng.copy(out, in_)
                    else:
                        eng.tensor_copy(out, in_)

                # ---- projection emitters (share the psS psum pool) ----
                def k_proj_sh(sh):
                    ceng = nc.vector
                    ps = psS.tile([128, 1024], f32, tag="S", name=f"kp{sh}")
                    for sub in range(2):
                        sc = 2 * sh + sub
                        for kc in range(KCT):
                            nc.tensor.matmul(
                                ps[:, 512 * sub:512 * (sub + 1)],
                                lhsT=wk[:, kc, :],
                                rhs=xT[:, kc, 512 * sc:512 * (sc + 1)],
                                start=(kc == 0), stop=(kc == KCT - 1))
                    sl = slice(1024 * sh, 1024 * (sh + 1))
                    _copy(ceng, kRep[0:64, 0, sl], ps[0:64, :])
                    _copy(ceng, kRep[64:128, 1, sl], ps[64:128, :])
                    nc.sync.dma_start(out=kRep[64:128, 0, sl],
                                      in_=kRep[0:64, 0, sl])
                    nc.sync.dma_start(out=kRep[0:64, 1, sl],
                                      in_=kRep[64:128, 1, sl])

                def q_half(p, h):
                    ceng = nc.vector
                    ps = psS.tile([128, 1024], f32, tag="S", name=f"qp{p}{h}")
                    for sub in range(2):
                        tcn = 2 * h + sub
                        for kc in range(KCT):
                            nc.tensor.matmul(
                                ps[:, 512 * sub:512 * (sub + 1)],
                                lhsT=wq[:, kc, 128 * p:128 * (p + 1)],
                                rhs=xT[:, kc, 512 * tcn:512 * (tcn + 1)],
                                start=(kc == 0), stop=(kc == KCT - 1))
                    sl = slice(1024 * h, 1024 * (h + 1))
                    _copy(ceng, qRep[0:64, 2 * p, sl], ps[0:64, :])
                    _copy(ceng, qRep[64:128, 2 * p + 1, sl], ps[64:128, :])
                    nc.sync.dma_start(out=qRep[64:128, 2 * p, sl],
                                      in_=qRep[0:64, 2 * p, sl])
                    nc.sync.dma_start(out=qRep[0:64, 2 * p + 1, sl],
                                      in_=qRep[64:128, 2 * p + 1, sl])

                def v_half(h):
                    ceng = nc.vector
                    ps = psS.tile([128, 1024], f32, tag="S", name=f"vh{h}")
                    for b in range(8):
                        st = 8 * h + b
                        for kc in range(KCT):
                            nc.tensor.matmul(
                                ps[:, 128 * b:128 * (b + 1)],
                                lhsT=xT[:, kc, 128 * st:128 * (st + 1)],
                                rhs=wv[:, kc, :],
                                start=(kc == 0), stop=(kc == KCT - 1))
                    ps3 = ps.rearrange("p (s d) -> p s d", d=128)
                    sl = slice(8 * h, 8 * (h + 1))
                    _copy(ceng, v_sb[:, sl, 0:64], ps3[:, :, 0:64])
                    _copy(ceng, v_sb[:, sl, 65:129], ps3[:, :, 64:128])
                    nc.vector.memset(v_sb[:, sl, 64], 1.0)
                    nc.vector.memset(v_sb[:, sl, 129], 1.0)

                RR = {}
                DMIN = {}

                def emit_scores(i, j, mid=None):
                    p, half = i // 2, i % 2
                    W, W5 = _W128[i][j], _W512[i][j]
                    lo = 128 * j         # t < lo : future region
                    hi = 128 * (j + 1)   # t >= hi: past region (Toeplitz)
                    nchunks = (W + 511) // 512
                    E = ebufp.tile([128, T], bf16, tag="E", name=f"E{i}_{j}")
                    for sh in range((nchunks + 1) // 2):
                        c0, c1 = 2 * sh, min(nchunks, 2 * sh + 2)
                        S = psS.tile([128, 1024], f32, tag="S",
                                     name=f"S{i}_{j}_{sh}")
                        for c in range(c0, c1):
                            rh = 64 * (c % 2)
                            o = 512 * (c - c0)
                            n = min(512, W - 512 * c)
                            nc.tensor.matmul(
                                S[:, o:o + n],
                                lhsT=kRep[rh:rh + 64, half,
                                          128 * j:128 * (j + 1)],
                                rhs=qRep[rh:rh + 64, i, 512 * c:512 * c + n],
                                start=True, stop=True)
                        wv_ = min(1024, W - 1024 * sh)
                        # chunks fully inside the future region need no alibi
                        # bias (it cancels against the u multiplier exactly)
                        full_future = 1024 * (sh + 1) <= lo
                        nc.scalar.activation(
                            E[:, 1024 * sh:1024 * sh + wv_], S[:, :wv_],
                            AF.Exp,
                            bias=0.0 if full_future else biassb[:, i:i + 1],
                            scale=0.125)
                        if sh == 0 and mid is not None:
                            # the pending attn@v streams on the PE between
                            # this tile's two score chunks, giving the ACT
                            # time to drain exp(sh1) before the PE needs its
                            # psum slot back (removes the per-step lockstep)
                            mid()
                            mid = None
                    if mid is not None:
                        mid()
                    if W5 > W and any(_J_FIRST[i][tcn] == j
                                      for tcn in range(NCH)):
                        nc.vector.memset(E[:, W:W5], 0.0)
                    # diag multiplier min(exp(-a(t_in-127)), exp(a(127-s_in)))
                    if i not in DMIN:
                        DMIN[i] = work.tile([128, 128], bf16, tag="dmin",
                                            name=f"dm{i}")
                        nc.vector.tensor_scalar(DMIN[i], wrep[:, i, 0:128],
                                                usb[:, i:i + 1], None, MIN)
                    lo0 = (lo // 1024) * 1024  # u-mult on the partial chunk
                    if lo > lo0:
                        nc.vector.tensor_scalar(E[:, lo0:lo], E[:, lo0:lo],
                                                usb[:, i:i + 1], None, MUL)
                    nc.vector.tensor_tensor(E[:, lo:hi], E[:, lo:hi], DMIN[i],
                                            MUL)
                    if W > hi:
                        nc.vector.tensor_tensor(
                            E[:, hi:W], E[:, hi:W],
                            wrep[:, i, 128:128 + (W - hi)], MUL)
                    return E

                def emit_av(i, j, pa, E):
                    half = i % 2
                    W = _W128[i][j]
                    for tcn in range(_W512[i][j] // 512):
                        first = j == _J_FIRST[i][tcn]
                        # the initializing tile streams the full zero-padded
                        # 512 so the psum region is defined; later tiles
                        # stream exact widths
                        n = 512 if first else min(512, W - 512 * tcn)
                        nc.tensor.matmul(
                            pa[:, 512 * tcn:512 * tcn + n],
                            lhsT=v_sb[:, j, 65 * half:65 * half + 65],
                            rhs=E[:, 512 * tcn:512 * tcn + n],
                            start=first, stop=(j == ST - 1),
                            skip_group_check=True)

                def emit_epilogue(i, pa):
                    p, half = i // 2, i % 2
                    st65 = stp.tile([65, T], bf16, tag="st65", name=f"st{i}")
                    # two half copies: the next head's first attn@v (psum
                    # region [0,512)) only WARs against the first half read
                    nc.vector.tensor_copy(st65[:, 0:1024], pa[0:65, 0:1024])
                    nc.vector.tensor_copy(st65[:, 1024:2048],
                                          pa[0:65, 1024:2048])
                    nc.sync.dma_start(out=outT[64 * half:64 * half + 64, p, :],
                                      in_=st65[0:64, :])
                    # denominator row -> [16,128] (partition-major reciprocal)
                    nc.sync.dma_start(out=drow_d[i:i + 1, :], in_=st65[64:65, :])
                    dsp = dpk.tile([16, 128], bf16, tag="dsp", name=f"dsp{i}")
                    nc.gpsimd.dma_start(
                        out=dsp, in_=drow_d[i].rearrange("(a b) -> a b", b=128))
                    rpf = dpk.tile([16, 128], f32, tag="rpf", name=f"rpf{i}")
                    nc.vector.reciprocal(rpf, dsp)
                    rp = dpk.tile([16, 128], bf16, tag="rp", name=f"rp{i}")
                    nc.vector.tensor_copy(rp, rpf)
                    nc.gpsimd.dma_start(
                        out=rrow_d[i].rearrange("(a b) -> a b", b=128), in_=rp)
                    if half == 0:
                        RR[p] = rrp.tile([128, T], bf16, tag="rr", name=f"rr{p}")
                    nc.gpsimd.dma_start(out=RR[p][64 * half:64 * half + 64, :],
                                        in_=bcast(rrow_d[i:i + 1, :], 64))
                    if half == 1:
                        nc.vector.tensor_tensor(outT[:, p, :], outT[:, p, :],
                                                RR[p], MUL)

                # ---- emission: software-pipelined attention stream ----
                # head 0 starts after only k(t<1024) + q(heads 0-1, t<1024);
                # the rest of the projections interleave into its j-loop
                k_proj_sh(0)
                q_half(0, 0)
                ilv = {
                    (0, 1): [lambda: v_half(0)],
                    (0, 2): [lambda: q_half(0, 1)],
                    (0, 3): [lambda: k_proj_sh(1)],
                    (0, 5): [lambda: v_half(1)],
                    (1, 4): [lambda: q_half(1, 0)],
                    (1, 8): [lambda: q_half(1, 1)],
                    (3, 4): [lambda: q_half(2, 0)],
                    (3, 8): [lambda: q_half(2, 1)],
                    (5, 4): [lambda: q_half(3, 0)],
                    (5, 8): [lambda: q_half(3, 1)],
                }
                pending = None   # (i, j, pa, E) awaiting attn@v emission
                epiq = []        # delayed epilogues: (steps_left, fn)
                for i in range(NH):
                    pa = psAp.tile([65, T], f32, tag="pa", name=f"pa{i}")
                    for j in range(ST):
                        for fn in ilv.get((i, j), ()):
                            fn()
                        # delayed epilogues fire BEFORE the attn@v flush: the
                        # epilogue of head i must be emitted before head i+1's
                        # first attn@v write reuses the psum accumulator
                        epiq = [(n - 1, fn) for n, fn in epiq]
                        for _, fn in [e for e in epiq if e[0] <= 0]:
                            fn()
                        epiq = [e for e in epiq if e[0] > 0]
                        mid = ((lambda pend=pending: emit_av(*pend))
                               if pending is not None else None)
                        E = emit_scores(i, j, mid)
                        pending = (i, j, pa, E)
                    if i < NH - 1:
                        # fire 2 pipeline steps into the next head so the
                        # epilogue DVE work queues behind the next head's
                        # multipliers
                        epiq.append((2, lambda pi=i, pp=pa:
                                     emit_epilogue(pi, pp)))
                pa_last = pending[2]
                emit_av(*pending)
                for _, fn in epiq:
                    fn()

                # ---- tail: the last head's epilogue is processed in 512-col
                # ---- quarters, each followed by its 4 output-projection
                # ---- tiles, so the PE streams the projection while the
                # ---- reciprocal chain of the next quarter is in flight
                oeng = [nc.sync, nc.scalar, nc.gpsimd]
                st65 = stp.tile([65, T], bf16, tag="st65", name="st7")
                rr16 = drow_d[NH - 1].rearrange("(a b) -> a b", b=128)
                rp16 = rrow_d[NH - 1].rearrange("(a b) -> a b", b=128)

                def pp_tile(tt):
                    pp = psS.tile([128, 1024], f32, tag="S", name=f"pp{tt}")
                    for ec in range(2):
                        for kt in range(4):
                            nc.tensor.matmul(
                                pp[:, 512 * ec:512 * (ec + 1)],
                                lhsT=outT[:, kt, 128 * tt:128 * (tt + 1)],
                                rhs=wp[:, kt, 512 * ec:512 * (ec + 1)],
                                start=(kt == 0), stop=(kt == 3))
                    osb = outp.tile([128, C], bf16, tag="osb", name=f"ob{tt}")
                    if tt % 2 == 0:
                        nc.scalar.copy(osb, pp)
                    else:
                        nc.vector.tensor_copy(osb, pp)
                    for ec in range(2):
                        oeng[(2 * tt + ec) % 3].dma_start(
                            out=out_d[128 * tt:128 * (tt + 1),
                                      512 * ec:512 * (ec + 1)],
                            in_=osb[:, 512 * ec:512 * (ec + 1)])

                for q in range(NCH):
                    ql = slice(512 * q, 512 * (q + 1))
                    nc.vector.tensor_copy(st65[:, ql], pa_last[0:65, ql])
                    nc.sync.dma_start(out=outT[64:128, 3, ql],
                                      in_=st65[0:64, ql])
                    nc.sync.dma_start(out=drow_d[NH - 1:NH, ql],
                                      in_=st65[64:65, ql])
                    dsp = dpk.tile([4, 128], bf16, tag="dsp", name=f"dspq{q}")
                    nc.gpsimd.dma_start(out=dsp, in_=rr16[4 * q:4 * q + 4, :])
                    rpf = dpk.tile([4, 128], f32, tag="rpf", name=f"rpfq{q}")
                    nc.vector.reciprocal(rpf, dsp)
                    rp = dpk.tile([4, 128], bf16, tag="rp", name=f"rpq{q}")
                    nc.vector.tensor_copy(rp, rpf)
                    nc.gpsimd.dma_start(out=rp16[4 * q:4 * q + 4, :], in_=rp)
                    nc.gpsimd.dma_start(
                        out=RR[3][64:128, ql],
                        in_=bcast(rrow_d[NH - 1:NH, ql], 64))
                    nc.vector.tensor_tensor(outT[:, 3, ql], outT[:, 3, ql],
                                            RR[3][:, ql], MUL)
                    for tt in range(4 * q, 4 * q + 4):
                        pp_tile(tt)

    _split_multiwait(nc, mybir)
    _NC_CACHE["nc"] = nc
    return nc


def _prep_core_inputs(x, Wq, Wkv, Wproj, b, g):
    import ml_dtypes
    bf = ml_dtypes.bfloat16
    heads = [_head_of_slot(i, g) for i in range(NH)]
    xT = np.ascontiguousarray(x[b].T).astype(bf)                      # [C, T]
    wq_cols = np.concatenate([Wq[64 * h:64 * (h + 1)] for h in heads], axis=0)
    wqT = np.ascontiguousarray(wq_cols.T).astype(bf)                  # [C, 512]
    wkT = np.ascontiguousarray(Wkv[128 * g:128 * (g + 1)].T).astype(bf)
    wvT = np.ascontiguousarray(Wkv[256 + 128 * g:256 + 128 * (g + 1)].T).astype(bf)
    cols = np.concatenate([np.arange(64 * h, 64 * (h + 1)) for h in heads])
    wpT = np.ascontiguousarray(Wproj[:, cols].T).astype(bf)           # [512, C]

    s_in = np.arange(128, dtype=np.float64)
    wrow = np.empty((NH, WREP_W), dtype=bf)
    u = np.empty((128, NH), dtype=np.float32)
    bias = np.empty((128, NH), dtype=np.float32)
    idx = np.arange(WREP_W, dtype=np.float64)
    for i, h in enumerate(heads):
        a = _a_of_head(h)
        wrow[i] = np.exp(-a * (idx - 127.0)).astype(np.float32)
        u[:, i] = np.exp(a * (127.0 - s_in)).astype(np.float32)
        bias[:, i] = (a * (s_in - 127.0)).astype(np.float32)
    return {"xT": xT, "wqT": wqT, "wkT": wkT, "wvT": wvT, "wpT": wpT,
            "wrow": wrow, "usb": u, "biassb": bias}


def kernel(x, Wq, Wkv, Wproj, bproj):
    from concourse.bass_utils import run_bass_kernel_spmd
    x = np.asarray(x, dtype=np.float32)
    Wq = np.asarray(Wq, dtype=np.float32)
    Wkv = np.asarray(Wkv, dtype=np.float32)
    Wproj = np.asarray(Wproj, dtype=np.float32)
    bproj = np.asarray(bproj, dtype=np.float32)

    nc = _build_nc()
    in_maps = [_prep_core_inputs(x, Wq, Wkv, Wproj, c // 2, c % 2)
               for c in range(8)]
    res = run_bass_kernel_spmd(nc, in_maps, core_ids=list(range(8)))
    out = np.zeros((B, T, C), dtype=np.float32)
    for c in range(8):
        out[c // 2] += np.asarray(res.results[c]["out"], dtype=np.float32)
    out += bproj[None, None, :]
    return out


# revision 54
# speedup vs baseline: 1.0162x; 1.0035x over previous
"""Bass/Tile TRN2 kernel for nn_DecoderGroupedQueryHeadAttentionAlibi.

Sharding (8 cores): core = (b, g) with b = core//2 in [0,4) (batch),
g = core%2 (head-half). Each core computes 8 of 16 query heads (those with
h%4 in {2g, 2g+1}) for its batch, plus the corresponding row-slice of the
output projection; the host sums the two half partials and adds bproj.

v3 changes over the baseline:
  - the j-loop is software-pipelined: attn@v for s-tile j is emitted after
    scores/exp/multipliers of s-tile j+1, so the PE streams the next tile's
    scores while the DVE finishes the current tile's row multipliers
    (removes a per-tile PE stall on the DVE).
  - x input DMA is split into t-halves so the first projections start
    after half the input has landed.
  - psum attn accumulator is split into two [65,1024] tiles so the next
    head's attn@v can start after half the epilogue copy.
  - per-head epilogue: direct [1,T] reciprocal + one DRAM-broadcast hop
    (replaces a 4-hop DMA round-trip chain).
  - output projection reuses the score PSUM pool (no pool-transition
    barrier), writes bf16 (summed on host in fp32), pipelines per t-tile,
    and spreads the output DMA over 3 queues.

Per-core device program (layout A, scoresT = [s_partitions, t_free]):
  per (head, s-tile): scoresT psum [128,1024] tiles -> ACT exp (alibi bias
  folded into the per-partition activation bias) -> DVE multiplier applied
  per region (past/diag/future) -> attn@v psum accumulation where row 64
  (a ones column in v) is the softmax denominator.

The alibi bias of this module is min(a_h*(s-t), 0) (tril overwrites the
causal mask in the torch reference, so future tokens are attended with bias
0), hence P = exp(score/8) * min(exp(a*(s-t)), 1), which factors into a
per-partition ACT bias exp(a*(s_in-127)) and a distance-only (Toeplitz) row
multiplier exp(-a*delta). Score columns with t - s > margin/a are dropped
(banded): the dropped softmax mass is < e^-margin of the kept band mass.
"""

import math
import numpy as np

# ---- problem constants (hardcoded; kernel.py must be self-contained) ----
B, T, C = 4, 2048, 1024
N_HEAD, N_KV_HEAD, HEAD_DIM = 16, 4, 64
NH = 8            # heads per core
ST = T // 128     # 16 s-tiles
NCH = T // 512    # 4 t-chunks
KCT = C // 128    # 8 contraction tiles of 128
WREP_W = 2048     # Toeplitz table width: index = t - 128*j is always < 2048
MARGIN = 4.5      # exp(-4.5): adds ~3e-3 relmax (validated), total ~9e-3 < 2e-2

_START = 2.0 ** (-2.0 ** (-(math.log2(N_HEAD) - 3.0)))  # 0.7071...


def _head_of_slot(i: int, g: int) -> int:
    return 4 * (i // 2) + 2 * g + (i % 2)


def _a_of_head(h: int) -> float:
    return (_START ** (h + 1)) / math.sqrt(HEAD_DIM)


# Loop bounds must be identical on every core (SPMD): use the widest cutoff
# over g for each head slot (g=1 heads have smaller slopes -> wider bands).
_CUTOFF = [MARGIN / min(_a_of_head(_head_of_slot(i, 0)),
                        _a_of_head(_head_of_slot(i, 1)))
           for i in range(NH)]
_R128 = [min(T, int(math.ceil(c / 128.0)) * 128) for c in _CUTOFF]
# 128-granular computed width per (head slot, s-tile): scores/exp/multiplies
_W128 = [[min(T, 128 * (j + 1) + _R128[i]) for j in range(ST)]
         for i in range(NH)]
# 512-granular width for the attn@v accumulation (E tail is zeroed)
_W512 = [[((w + 511) // 512) * 512 for w in row] for row in _W128]
_NEFF = [[w // 512 for w in row] for row in _W512]
_J_FIRST = [[min(j for j in range(ST) if _NEFF[i][j] > tcn)
             for tcn in range(NCH)] for i in range(NH)]

_NC_CACHE = {}


def _split_multiwait(nc, mybir, max_waits=1):
    """walrus in this env encodes at most one sync-wait per instruction;
    split extras onto same-engine NoOps emitted just before."""
    for f in nc.m.functions:
        for bb in f.blocks:
            new = []
            for ins in bb.instructions:
                si = ins.sync_info
                conds = list(si.on_wait) if si is not None else []
                if len(conds) > max_waits:
                    for cond in conds[:-max_waits]:
                        n = mybir.InstNoOp(
                            name=nc.get_next_instruction_name(), ins=[], outs=[])
                        n.engine = ins.engine
                        n.sync_info = mybir.SyncInfo(on_wait=[cond], on_update=[])
                        new.append(n)
                    si.on_wait = conds[-max_waits:]
                new.append(ins)
            bb.instructions = new


def _build_nc():
    if "nc" in _NC_CACHE:
        return _NC_CACHE["nc"]
    import concourse.bass as bass
    import concourse.tile as tile
    from concourse import mybir

    f32 = mybir.dt.float32
    bf16 = mybir.dt.bfloat16
    AF = mybir.ActivationFunctionType
    MUL = mybir.AluOpType.mult
    MIN = mybir.AluOpType.min

    nc = bass.Bass()

    xT_d = nc.dram_tensor("xT", [C, T], bf16, kind="ExternalInput")
    wq_d = nc.dram_tensor("wqT", [C, NH * 64], bf16, kind="ExternalInput")
    wk_d = nc.dram_tensor("wkT", [C, 128], bf16, kind="ExternalInput")
    wv_d = nc.dram_tensor("wvT", [C, 128], bf16, kind="ExternalInput")
    wp_d = nc.dram_tensor("wpT", [NH * 64, C], bf16, kind="ExternalInput")
    wrow_d = nc.dram_tensor("wrow", [NH, WREP_W], bf16, kind="ExternalInput")
    u_d = nc.dram_tensor("usb", [128, NH], f32, kind="ExternalInput")
    bias_d = nc.dram_tensor("biassb", [128, NH], f32, kind="ExternalInput")
    out_d = nc.dram_tensor("out", [T, C], bf16, kind="ExternalOutput")

    xT_r = xT_d.rearrange("(k p) t -> p k t", p=128)
    wq_r = wq_d.rearrange("(k p) e -> p k e", p=128)

    def bcast(src_row, parts):
        # [1, W] DRAM row -> [parts, W] stride-0 partition broadcast source
        return bass.AP(tensor=src_row.tensor, offset=src_row.offset,
                       ap=[[0, parts]] + list(src_row.ap)[1:])

    with tile.TileContext(nc) as tc:
        with (
            tc.tile_pool(name="const", bufs=1) as const,
            tc.tile_pool(name="work", bufs=3) as work,
            tc.tile_pool(name="ebuf", bufs=4) as ebufp,
            tc.tile_pool(name="stp", bufs=2) as stp,
            tc.tile_pool(name="rrp", bufs=2) as rrp,
            tc.tile_pool(name="dpk", bufs=4) as dpk,
            tc.tile_pool(name="outp", bufs=8) as outp,
            tc.tile_pool(name="dramd", bufs=1, space="DRAM") as dramd,
        ):
            # ---- persistent tiles ----
            kRep = const.tile([128, 2, T], bf16)     # kv on both halves
            v_sb = const.tile([128, ST, 130], bf16)  # [s, j, (v_kv0|1|v_kv1|1)]
            qRep = const.tile([128, NH, T], bf16)    # head i on both halves
            outT = const.tile([128, 4, T], bf16)     # [(2 heads d), pair, t]
            wrep = const.tile([128, NH, WREP_W], bf16)
            wp = const.tile([128, 4, C], bf16)
            usb = const.tile([128, NH], f32)
            biassb = const.tile([128, NH], f32)
            xT = const.tile([128, KCT, T], bf16)
            wq = const.tile([128, KCT, NH * 64], bf16)
            wk = const.tile([128, KCT, 128], bf16)
            wv = const.tile([128, KCT, 128], bf16)
            warm = const.tile([128, 1], f32)
            wsink = const.tile([128, 1], f32)
            drow_d = dramd.tile([NH, T], bf16)
            rrow_d = dramd.tile([NH, T], bf16)

            # ---- ACT exp-table preload (runs during the DMA ramp) ----
            nc.vector.memset(warm, 0.0)
            nc.scalar.activation(wsink, warm, AF.Exp, scale=1.0)

            # ---- input DMAs; x lands in t-order (first 512 cols of every
            # ---- contraction chunk first) so the projections start early
            for sc in range(2):
                for kc in range(KCT):
                    eng = nc.sync if kc % 2 == 0 else nc.scalar
                    eng.dma_start(out=xT[:, kc, 512 * sc:512 * (sc + 1)],
                                  in_=xT_r[:, kc, 512 * sc:512 * (sc + 1)])
            for kc in range(KCT):
                eng = nc.sync if kc % 2 == 0 else nc.scalar
                eng.dma_start(out=xT[:, kc, 1024:2048],
                              in_=xT_r[:, kc, 1024:2048])
            nc.gpsimd.dma_start(out=wk, in_=wk_d.rearrange("(k p) e -> p k e", p=128))
            nc.gpsimd.dma_start(out=usb, in_=u_d[:])
            nc.gpsimd.dma_start(out=biassb, in_=bias_d[:])
            wrow_r = [wrow_d[i:i + 1, :] for i in range(NH)]

            def wrep_bc(i):
                nc.gpsimd.dma_start(out=wrep[:, i, :], in_=bcast(wrow_r[i], 128))

            wrep_bc(0)
            wrep_bc(1)
            for kc in range(KCT):
                nc.gpsimd.dma_start(out=wq[:, kc, :], in_=wq_r[:, kc, :])
            nc.gpsimd.dma_start(out=wv, in_=wv_d.rearrange("(k p) e -> p k e", p=128))
            for i in range(2, NH):
                wrep_bc(i)
            nc.gpsimd.dma_start(out=wp, in_=wp_d.rearrange("(k p) e -> p k e", p=128))

            with (
                tc.tile_pool(name="psS", bufs=2, space="PSUM") as psS,
                tc.tile_pool(name="psA", bufs=1, space="PSUM") as psAp,
            ):
                def _copy(eng, out, in_):
                    if eng is nc.scalar:
                        eng.copy(out, in_)
                    else:
                        eng.tensor_copy(out, in_)

                # ---- projection emitters (share the psS psum pool) ----
                def k_proj_sh(sh):
                    ceng = nc.vector
                    ps = psS.tile([128, 1024], f32, tag="S", name=f"kp{sh}")
                    for sub in range(2):
                        sc = 2 * sh + sub
                        for kc in range(KCT):
                            nc.tensor.matmul(
                                ps[:, 512 * sub:512 * (sub + 1)],
                                lhsT=wk[:, kc, :],
                                rhs=xT[:, kc, 512 * sc:512 * (sc + 1)],
                                start=(kc == 0), stop=(kc == KCT - 1))
                    sl = slice(1024 * sh, 1024 * (sh + 1))
                    _copy(ceng, kRep[0:64, 0, sl], ps[0:64, :])
                    _copy(ceng, kRep[64:128, 1, sl], ps[64:128, :])
                    nc.sync.dma_start(out=kRep[64:128, 0, sl],
                                      in_=kRep[0:64, 0, sl])
                    nc.sync.dma_start(out=kRep[0:64, 1, sl],
                                      in_=kRep[64:128, 1, sl])

                def q_half(p, h):
                    ceng = nc.vector
                    ps = psS.tile([128, 1024], f32, tag="S", name=f"qp{p}{h}")
                    for sub in range(2):
                        tcn = 2 * h + sub
                        for kc in range(KCT):
                            nc.tensor.matmul(
                                ps[:, 512 * sub:512 * (sub + 1)],
                                lhsT=wq[:, kc, 128 * p:128 * (p + 1)],
                                rhs=xT[:, kc, 512 * tcn:512 * (tcn + 1)],
                                start=(kc == 0), stop=(kc == KCT - 1))
                    sl = slice(1024 * h, 1024 * (h + 1))
                    _copy(ceng, qRep[0:64, 2 * p, sl], ps[0:64, :])
                    _copy(ceng, qRep[64:128, 2 * p + 1, sl], ps[64:128, :])
                    nc.sync.dma_start(out=qRep[64:128, 2 * p, sl],
                                      in_=qRep[0:64, 2 * p, sl])
                    nc.sync.dma_start(out=qRep[0:64, 2 * p + 1, sl],
                                      in_=qRep[64:128, 2 * p + 1, sl])

                def v_half(h):
                    ceng = nc.vector
                    ps = psS.tile([128, 1024], f32, tag="S", name=f"vh{h}")
                    for b in range(8):
                        st = 8 * h + b
                        for kc in range(KCT):
                            nc.tensor.matmul(
                                ps[:, 128 * b:128 * (b + 1)],
                                lhsT=xT[:, kc, 128 * st:128 * (st + 1)],
                                rhs=wv[:, kc, :],
                                start=(kc == 0), stop=(kc == KCT - 1))
                    ps3 = ps.rearrange("p (s d) -> p s d", d=128)
                    sl = slice(8 * h, 8 * (h + 1))
                    _copy(ceng, v_sb[:, sl, 0:64], ps3[:, :, 0:64])
                    _copy(ceng, v_sb[:, sl, 65:129], ps3[:, :, 64:128])
                    nc.vector.memset(v_sb[:, sl, 64], 1.0)
                    nc.vector.memset(v_sb[:, sl, 129], 1.0)

                RR = {}
                DMIN = {}

                def emit_scores(i, j, mid=None):
                    p, half = i // 2, i % 2
                    W, W5 = _W128[i][j], _W512[i][j]
                    lo = 128 * j         # t < lo : future region
                    hi = 128 * (j + 1)   # t >= hi: past region (Toeplitz)
                    nchunks = (W + 511) // 512
                    E = ebufp.tile([128, T], bf16, tag="E", name=f"E{i}_{j}")
                    for sh in range((nchunks + 1) // 2):
                        c0, c1 = 2 * sh, min(nchunks, 2 * sh + 2)
                        S = psS.tile([128, 1024], f32, tag="S",
                                     name=f"S{i}_{j}_{sh}")
                        for c in range(c0, c1):
                            rh = 64 * (c % 2)
                            o = 512 * (c - c0)
                            n = min(512, W - 512 * c)
                            nc.tensor.matmul(
                                S[:, o:o + n],
                                lhsT=kRep[rh:rh + 64, half,
                                          128 * j:128 * (j + 1)],
                                rhs=qRep[rh:rh + 64, i, 512 * c:512 * c + n],
                                start=True, stop=True)
                        wv_ = min(1024, W - 1024 * sh)
                        # chunks fully inside the future region need no alibi
                        # bias (it cancels against the u multiplier exactly)
                        full_future = 1024 * (sh + 1) <= lo
                        nc.scalar.activation(
                            E[:, 1024 * sh:1024 * sh + wv_], S[:, :wv_],
                            AF.Exp,
                            bias=0.0 if full_future else biassb[:, i:i + 1],
                            scale=0.125)
                        if sh == 0 and mid is not None:
                            # the pending attn@v streams on the PE between
                            # this tile's two score chunks, giving the ACT
                            # time to drain exp(sh1) before the PE needs its
                            # psum slot back (removes the per-step lockstep)
                            mid()
                            mid = None
                    if mid is not None:
                        mid()
                    if W5 > W and any(_J_FIRST[i][tcn] == j
                                      for tcn in range(NCH)):
                        nc.vector.memset(E[:, W:W5], 0.0)
                    # diag multiplier min(exp(-a(t_in-127)), exp(a(127-s_in)))
                    if i not in DMIN:
                        DMIN[i] = work.tile([128, 128], bf16, tag="dmin",
                                            name=f"dm{i}")
                        nc.vector.tensor_scalar(DMIN[i], wrep[:, i, 0:128],
                                                usb[:, i:i + 1], None, MIN)
                    lo0 = (lo // 1024) * 1024  # u-mult on the partial chunk
                    if lo > lo0:
                        nc.vector.tensor_scalar(E[:, lo0:lo], E[:, lo0:lo],
                                                usb[:, i:i + 1], None, MUL)
                    nc.vector.tensor_tensor(E[:, lo:hi], E[:, lo:hi], DMIN[i],
                                            MUL)
                    if W > hi:
                        nc.vector.tensor_tensor(
                            E[:, hi:W], E[:, hi:W],
                            wrep[:, i, 128:128 + (W - hi)], MUL)
                    return E

                def emit_av(i, j, pa, E):
                    half = i % 2
                    W = _W128[i][j]
                    for tcn in range(_W512[i][j] // 512):
                        first = j == _J_FIRST[i][tcn]
                        # the initializing tile streams the full zero-padded
                        # 512 so the psum region is defined; later tiles
                        # stream exact widths
                        n = 512 if first else min(512, W - 512 * tcn)
                        nc.tensor.matmul(
                            pa[:, 512 * tcn:512 * tcn + n],
                            lhsT=v_sb[:, j, 65 * half:65 * half + 65],
                            rhs=E[:, 512 * tcn:512 * tcn + n],
                            start=first, stop=(j == ST - 1),
                            skip_group_check=True)

                def emit_epilogue(i, pa):
                    p, half = i // 2, i % 2
                    st65 = stp.tile([65, T], bf16, tag="st65", name=f"st{i}")
                    nc.vector.tensor_copy(st65, pa[0:65, :])
                    nc.sync.dma_start(out=outT[64 * half:64 * half + 64, p, :],
                                      in_=st65[0:64, :])
                    # denominator row -> [16,128] (partition-major reciprocal)
                    nc.sync.dma_start(out=drow_d[i:i + 1, :], in_=st65[64:65, :])
                    dsp = dpk.tile([16, 128], bf16, tag="dsp", name=f"dsp{i}")
                    nc.gpsimd.dma_start(
                        out=dsp, in_=drow_d[i].rearrange("(a b) -> a b", b=128))
                    rpf = dpk.tile([16, 128], f32, tag="rpf", name=f"rpf{i}")
                    nc.vector.reciprocal(rpf, dsp)
                    rp = dpk.tile([16, 128], bf16, tag="rp", name=f"rp{i}")
                    nc.vector.tensor_copy(rp, rpf)
                    nc.gpsimd.dma_start(
                        out=rrow_d[i].rearrange("(a b) -> a b", b=128), in_=rp)
                    if half == 0:
                        RR[p] = rrp.tile([128, T], bf16, tag="rr", name=f"rr{p}")
                    nc.gpsimd.dma_start(out=RR[p][64 * half:64 * half + 64, :],
                                        in_=bcast(rrow_d[i:i + 1, :], 64))
                    if half == 1:
                        nc.vector.tensor_tensor(outT[:, p, :], outT[:, p, :],
                                                RR[p], MUL)

                # ---- emission: software-pipelined attention stream ----
                # head 0 starts after only k(t<1024) + q(heads 0-1, t<1024);
                # the rest of the projections interleave into its j-loop
                k_proj_sh(0)
                q_half(0, 0)
                ilv = {
                    (0, 1): [lambda: v_half(0)],
                    (0, 2): [lambda: q_half(0, 1)],
                    (0, 3): [lambda: k_proj_sh(1)],
                    (0, 5): [lambda: v_half(1)],
                    (1, 4): [lambda: q_half(1, 0)],
                    (1, 8): [lambda: q_half(1, 1)],
                    (3, 4): [lambda: q_half(2, 0)],
                    (3, 8): [lambda: q_half(2, 1)],
                    (5, 4): [lambda: q_half(3, 0)],
                    (5, 8): [lambda: q_half(3, 1)],
                }
                pending = None   # (i, j, pa, E) awaiting attn@v emission
                epiq = []        # delayed epilogues: (steps_left, fn)
                for i in range(NH):
                    pa = psAp.tile([65, T], f32, tag="pa", name=f"pa{i}")
                    for j in range(ST):
                        for fn in ilv.get((i, j), ()):
                            fn()
                        # delayed epilogues fire BEFORE the attn@v flush: the
                        # epilogue of head i must be emitted before head i+1's
                        # first attn@v write reuses the psum accumulator
                        epiq = [(n - 1, fn) for n, fn in epiq]
                        for _, fn in [e for e in epiq if e[0] <= 0]:
                            fn()
                        epiq = [e for e in epiq if e[0] > 0]
                        mid = ((lambda pend=pending: emit_av(*pend))
                               if pending is not None else None)
                        E = emit_scores(i, j, mid)
                        pending = (i, j, pa, E)
                    if i < NH - 1:
                        # fire 2 pipeline steps into the next head so the
                        # epilogue DVE work queues behind the next head's
                        # multipliers
                        epiq.append((2, lambda pi=i, pp=pa:
                                     emit_epilogue(pi, pp)))
                pa_last = pending[2]
                emit_av(*pending)
                for _, fn in epiq:
                    fn()

                # ---- tail: the last head's epilogue is processed in 512-col
                # ---- quarters, each followed by its 4 output-projection
                # ---- tiles, so the PE streams the projection while the
                # ---- reciprocal chain of the next quarter is in flight
                oeng = [nc.sync, nc.scalar, nc.gpsimd]
                st65 = stp.tile([65, T], bf16, tag="st65", name="st7")
                rr16 = drow_d[NH - 1].rearrange("(a b) -> a b", b=128)
                rp16 = rrow_d[NH - 1].rearrange("(a b) -> a b", b=128)

                def pp_tile(tt):
                    pp = psS.tile([128, 1024], f32, tag="S", name=f"pp{tt}")
                    for ec in range(2):
                        for kt in range(4):
                            nc.tensor.matmul(
                                pp[:, 512 * ec:512 * (ec + 1)],
                                lhsT=outT[:, kt, 128 * tt:128 * (tt + 1)],
                                rhs=wp[:, kt, 512 * ec:512 * (ec + 1)],
                                start=(kt == 0), stop=(kt == 3))
                    osb = outp.tile([128, C], bf16, tag="osb", name=f"ob{tt}")
                    if tt % 2 == 0:
                        nc.scalar.copy(osb, pp)
                    else:
                        nc.vector.tensor_copy(osb, pp)
                    for ec in range(2):
                        oeng[(2 * tt + ec) % 3].dma_start(
                            out=out_d[128 * tt:128 * (tt + 1),
                                      512 * ec:512 * (ec + 1)],
                            in_=osb[:, 512 * ec:512 * (ec + 1)])

                for q in range(NCH):
                    ql = slice(512 * q, 512 * (q + 1))
                    nc.vector.tensor_copy(st65[:, ql], pa_last[0:65, ql])
                    nc.sync.dma_start(out=outT[64:128, 3, ql],
                                      in_=st65[0:64, ql])
                    nc.sync.dma_start(out=drow_d[NH - 1:NH, ql],
                                      in_=st65[64:65, ql])
                    dsp = dpk.tile([4, 128], bf16, tag="dsp", name=f"dspq{q}")
                    nc.gpsimd.dma_start(out=dsp, in_=rr16[4 * q:4 * q + 4, :])
                    rpf = dpk.tile([4, 128], f32, tag="rpf", name=f"rpfq{q}")
                    nc.vector.reciprocal(rpf, dsp)
                    rp = dpk.tile([4, 128], bf16, tag="rp", name=f"rpq{q}")
                    nc.vector.tensor_copy(rp, rpf)
                    nc.gpsimd.dma_start(out=rp16[4 * q:4 * q + 4, :], in_=rp)
                    nc.gpsimd.dma_start(
                        out=RR[3][64:128, ql],
                        in_=bcast(rrow_d[NH - 1:NH, ql], 64))
                    nc.vector.tensor_tensor(outT[:, 3, ql], outT[:, 3, ql],
                                            RR[3][:, ql], MUL)
                    for tt in range(4 * q, 4 * q + 4):
                        pp_tile(tt)

    _split_multiwait(nc, mybir)
    _NC_CACHE["nc"] = nc
    return nc


def _prep_core_inputs(x, Wq, Wkv, Wproj, b, g):
    import ml_dtypes
    bf = ml_dtypes.bfloat16
    heads = [_head_of_slot(i, g) for i in range(NH)]
    xT = np.ascontiguousarray(x[b].T).astype(bf)                      # [C, T]
    wq_cols = np.concatenate([Wq[64 * h:64 * (h + 1)] for h in heads], axis=0)
    wqT = np.ascontiguousarray(wq_cols.T).astype(bf)                  # [C, 512]
    wkT = np.ascontiguousarray(Wkv[128 * g:128 * (g + 1)].T).astype(bf)
    wvT = np.ascontiguousarray(Wkv[256 + 128 * g:256 + 128 * (g + 1)].T).astype(bf)
    cols = np.concatenate([np.arange(64 * h, 64 * (h + 1)) for h in heads])
    wpT = np.ascontiguousarray(Wproj[:, cols].T).astype(bf)           # [512, C]

    s_in = np.arange(128, dtype=np.float64)
    wrow = np.empty((NH, WREP_W), dtype=bf)
    u = np.empty((128, NH), dtype=np.float32)
    bias = np.empty((128, NH), dtype=np.float32)
    idx = np.arange(WREP_W, dtype=np.float64)
    for i, h in enumerate(heads):
        a = _a_of_head(h)
        wrow[i] = np.exp(-a * (idx - 127.0)).astype(np.float32)
        u[:, i] = np.exp(a * (127.0 - s_in)).astype(np.float32)
        bias[:, i] = (a * (s_in - 127.0)).astype(np.float32)
    return {"xT": xT, "wqT": wqT, "wkT": wkT, "wvT": wvT, "wpT": wpT,
            "wrow": wrow, "usb": u, "biassb": bias}


def kernel(x, Wq, Wkv, Wproj, bproj):
    from concourse.bass_utils import run_bass_kernel_spmd
    x = np.asarray(x, dtype=np.float32)
    Wq = np.asarray(Wq, dtype=np.float32)
    Wkv = np.asarray(Wkv, dtype=np.float32)
    Wproj = np.asarray(Wproj, dtype=np.float32)
    bproj = np.asarray(bproj, dtype=np.float32)

    nc = _build_nc()
    in_maps = [_prep_core_inputs(x, Wq, Wkv, Wproj, c // 2, c % 2)
               for c in range(8)]
    res = run_bass_kernel_spmd(nc, in_maps, core_ids=list(range(8)))
    out = np.zeros((B, T, C), dtype=np.float32)
    for c in range(8):
        out[c // 2] += np.asarray(res.results[c]["out"], dtype=np.float32)
    out += bproj[None, None, :]
    return out
